# revision 9
# baseline (speedup 1.0000x reference)
"""GATNet (4-layer GAT, 10000 nodes / 50000 edges + self-loops) on 8 Trainium2 NeuronCores.

Self-contained: builds per-core shards on the host (edge bucketing by destination,
one-hot scatter masks, gather index tables), compiles one SPMD Bass program, runs it
on cores 0-7 via run_bass_kernel_spmd, and reassembles the full [10000, 1000] output.

Structure per layer:
  dense h = y @ W^T (bf16, attention projections folded as extra rhs columns)
  -> small AllGather of the per-node attention scores (al) + big AllGather of h;
     the whole softmax pre-phase (score gathers, e-values, segment denominators
     via one-hot matmuls, reciprocals, alphas) runs under the big AllGather
  -> gather phase: dma_gather of source h rows; one-hot scatter matmuls with the
     per-edge alpha folded into the stationary operand; bias via K=1 matmul
  -> ReLU eviction with fused row sums, graph-LayerNorm stats via tiny AllReduce,
     PE-transpose into feature-major for the next layer's lhsT.
Layer 1 never materializes h: by linearity sum_e alpha_e * (x W)[src_e] =
(sum_e alpha_e x[src_e]) W, so it scatters raw x rows (256 wide) and applies W1
once per destination window; its attention scores are computed exactly on the host.
"""
import sys
import types

import numpy as np
import ml_dtypes

BF16 = ml_dtypes.bfloat16

N_NODES = 10000
N_CORES = 8
NPC = 1250
NPAD = 1280
NT = 10
NW = 10
HEADS_L = [8, 8, 8, 1]
C_L = [448, 384, 256, 1000]
FIN_L = [256, 3584, 3072, 2048]
FOUT_L = [3584, 3072, 2048, 1000]
TCOL_L = [256, 3072, 2048, 1024]    # bf16 columns of the gather table (L1: raw x)
EXP_CLAMP = 35.0
DEN_TINY = 1e-30


def _install_ntff_hook():
    if "antenv.axon_hooks" in sys.modules:
        return
    try:
        import antenv
        from trn_agent_boot.trn_boot import _ntff_profile_via_ctypes
    except ImportError:
        return
    mod = types.ModuleType("antenv.axon_hooks")
    state = {"hook": None}
    mod.set_axon_ntff_profile_hook = lambda h: state.__setitem__("hook", h)
    mod.get_axon_ntff_profile_hook = lambda: state["hook"]
    sys.modules["antenv.axon_hooks"] = mod
    antenv.axon_hooks = mod
    mod.set_axon_ntff_profile_hook(_ntff_profile_via_ctypes("/opt/axon/libaxon_pjrt.so"))


# ---------------------------------------------------------------- host prep
def _table_row(n):
    return NPAD * (n // NPC) + (n % NPC)


def _wrap16(idx_chunk):
    w = idx_chunk.reshape(8, 16).T
    return np.tile(w, (8, 1)).astype(np.int16)


def prep_edges(edge_index):
    src = np.asarray(edge_index[0], dtype=np.int64)
    dst = np.asarray(edge_index[1], dtype=np.int64)
    src = np.concatenate([src, np.arange(N_NODES, dtype=np.int64)])
    dst = np.concatenate([dst, np.arange(N_NODES, dtype=np.int64)])

    buckets = [[[] for _ in range(NW)] for _ in range(N_CORES)]
    core_of = dst // NPC
    win_of = (dst % NPC) // 128
    order = np.argsort(dst, kind="stable")
    for e in order:
        buckets[core_of[e]][win_of[e]].append(e)

    ncw = []
    for w in range(NW):
        mx = max(len(buckets[k][w]) for k in range(N_CORES))
        ncw.append(max(1, -(-mx // 128)))
    nch = sum(ncw)

    per_core = []
    for k in range(N_CORES):
        idx_s = np.zeros((128, nch * 8), np.int16)
        idx_d = np.zeros((128, nch * 8), np.int16)
        mask = np.zeros((128, nch, 128), np.float32)
        maskT = np.zeros((128, nch, 128), np.float32)
        c0 = 0
        for w in range(NW):
            edges = buckets[k][w]
            for c in range(ncw[w]):
                part = edges[c * 128:(c + 1) * 128]
                srows = np.zeros(128, np.int64)
                drows = np.zeros(128, np.int64)
                for i, e in enumerate(part):
                    srows[i] = _table_row(src[e])
                    drows[i] = _table_row(dst[e])
                    d_local = (dst[e] % NPC) - 128 * w
                    mask[i, c0 + c, d_local] = 1.0
                    maskT[d_local, c0 + c, i] = 1.0
                idx_s[:, (c0 + c) * 8:(c0 + c + 1) * 8] = _wrap16(srows)
                idx_d[:, (c0 + c) * 8:(c0 + c + 1) * 8] = _wrap16(drows)
            c0 += ncw[w]
        per_core.append(dict(
            idxs=idx_s, idxd=idx_d,
            mask=mask.reshape(128, nch * 128).astype(BF16),
            maskT=maskT.reshape(128, nch * 128).astype(BF16),
        ))
    return tuple(ncw), per_core


def prep_params(inputs):
    p = {}
    x64 = np.asarray(inputs["x"], np.float64)
    al1 = None
    for li in range(4):
        H, C, fin, fout = HEADS_L[li], C_L[li], FIN_L[li], FOUT_L[li]
        W = np.asarray(inputs[f"W{li+1}"], np.float32)
        a_src = np.asarray(inputs[f"a_src{li+1}"], np.float32)
        a_dst = np.asarray(inputs[f"a_dst{li+1}"], np.float32)
        a_blk_s = np.zeros((fout, H), np.float32)
        a_blk_d = np.zeros((fout, H), np.float32)
        for h in range(H):
            a_blk_s[h * C:(h + 1) * C, h] = a_src[h]
            a_blk_d[h * C:(h + 1) * C, h] = a_dst[h]
        rhs = np.concatenate([W.T, W.T @ a_blk_s, W.T @ a_blk_d], axis=1)
        p[f"rhs{li+1}"] = np.ascontiguousarray(rhs).astype(BF16)
        p[f"brow{li+1}"] = np.asarray(inputs[f"b{li+1}"], np.float32).reshape(1, fout).astype(BF16)
        if li < 3:
            lw = np.asarray(inputs[f"ln{li+1}_w"], np.float32)
            lb = np.asarray(inputs[f"ln{li+1}_b"], np.float32)
            p[f"lnw{li+1}"] = np.ascontiguousarray(lw.reshape(fout // 128, 128).T)
            p[f"lnb{li+1}"] = np.ascontiguousarray(lb.reshape(fout // 128, 128).T)
        if li == 0:
            al1 = np.concatenate(
                [x64 @ (W.T @ a_blk_s).astype(np.float64),
                 x64 @ (W.T @ a_blk_d).astype(np.float64)], axis=1).astype(np.float32)
    # packed x table, replicated: [10240 rows, 768 bytes] = 512B x bf16 + 256B f32 al
    xtab = np.zeros((N_CORES * NPAD, 768), np.uint8)
    rows = _table_row(np.arange(N_NODES))
    xb = np.asarray(inputs["x"], np.float32).astype(BF16)
    xtab[rows, :512] = xb.view(np.uint8)
    alpad = np.zeros((N_NODES, 64), np.float32)
    alpad[:, :16] = al1
    xtab[rows, 512:768] = alpad.view(np.uint8)
    p["xtab"] = xtab.view(BF16)
    p["ident"] = np.eye(128, dtype=BF16)
    ones_b = np.zeros((1, NW * 128), np.float32)
    ones_b[0, :NPC] = 1.0
    p["ones_b"] = ones_b.astype(BF16)
    return p


# ---------------------------------------------------------------- device build
_CACHE = {}


def build(ncw, debug=False):
    key = (tuple(ncw), debug)
    if key in _CACHE:
        return _CACHE[key]

    import concourse.bacc as bacc
    import concourse.mybir as mybir
    import concourse.tile as tile
    from concourse.library_config import mlp

    f32 = mybir.dt.float32
    bf16 = mybir.dt.bfloat16
    i16 = mybir.dt.int16
    AX = mybir.AxisListType
    ALU = mybir.AluOpType
    ACTF = mybir.ActivationFunctionType

    nch = sum(ncw)
    ncmax = max(ncw)
    coff = [0]
    for w in range(NW):
        coff.append(coff[-1] + ncw[w])
    nc = bacc.Bacc("TRN2")

    xtab_d = nc.declare_dram_parameter("xtab", [N_CORES * NPAD, 384], bf16, isOutput=False)
    rhs_d, brow_d, lnw_d, lnb_d = [], [], [], []
    for li in range(4):
        H, fout, fin = HEADS_L[li], FOUT_L[li], FIN_L[li]
        rhs_d.append(nc.declare_dram_parameter(f"rhs{li+1}", [fin, fout + 2 * H], bf16, isOutput=False))
        brow_d.append(nc.declare_dram_parameter(f"brow{li+1}", [1, fout], bf16, isOutput=False))
        if li < 3:
            lnw_d.append(nc.declare_dram_parameter(f"lnw{li+1}", [128, fout // 128], f32, isOutput=False))
            lnb_d.append(nc.declare_dram_parameter(f"lnb{li+1}", [128, fout // 128], f32, isOutput=False))
    idxs_d = nc.declare_dram_parameter("idxs", [128, nch * 8], i16, isOutput=False)
    idxd_d = nc.declare_dram_parameter("idxd", [128, nch * 8], i16, isOutput=False)
    mask_d = nc.declare_dram_parameter("mask", [128, nch * 128], bf16, isOutput=False)
    maskT_d = nc.declare_dram_parameter("maskT", [128, nch * 128], bf16, isOutput=False)
    ones_d = nc.declare_dram_parameter("ones_b", [1, NW * 128], bf16, isOutput=False)
    ident_d = nc.declare_dram_parameter("ident", [128, 128], bf16, isOutput=False)
    out_d = nc.declare_dram_parameter("out", [NPC, 1000], f32, isOutput=True)
    dbg_zt, dbg_st = [], []
    if debug:
        for li in range(3):
            dbg_zt.append(nc.declare_dram_parameter(
                f"dbg_zt{li}", [128, 28 * NPAD], bf16, isOutput=True))
            dbg_st.append(nc.declare_dram_parameter(
                f"dbg_st{li}", [1, 8], f32, isOutput=True))

    RG = [list(range(N_CORES))]

    with tile.TileContext(nc) as tc:
        with (
            tc.tile_pool(name="const", bufs=1) as constp,
            tc.tile_pool(name="yt", bufs=1) as ytp,
            tc.tile_pool(name="rhs", bufs=5) as rhsp,
            tc.tile_pool(name="stage", bufs=2) as stagep,
            tc.tile_pool(name="gath", bufs=2) as gathp,
            tc.tile_pool(name="mw", bufs=2) as mwp,
            tc.tile_pool(name="eph", bufs=2) as ephp,
            tc.tile_pool(name="z", bufs=2) as zp,
            tc.tile_pool(name="misc", bufs=2) as miscp,
            tc.tile_pool(name="dram", bufs=1, space="DRAM") as dram,
        ):
            nc.gpsimd.load_library(mlp)

            idxs_t = constp.tile([128, nch, 8], i16, tag="idxs")
            nc.sync.dma_start(idxs_t[:], idxs_d[:].rearrange("p (c d) -> p c d", c=nch))
            idxd_t = constp.tile([128, nch, 8], i16, tag="idxd")
            nc.sync.dma_start(idxd_t[:], idxd_d[:].rearrange("p (c d) -> p c d", c=nch))
            ones_t = constp.tile([1, NW, 128], bf16, tag="onesb")
            nc.sync.dma_start(ones_t[:], ones_d[:].rearrange("p (w d) -> p w d", w=NW))
            ident_t = constp.tile([128, 128], bf16, tag="ident")
            nc.sync.dma_start(ident_t[:], ident_d[:])
            ones128 = constp.tile([128, 1], f32, tag="ones128")
            nc.vector.memset(ones128[:], 1.0)
            onesT = constp.tile([1, 128], f32, tag="onesT")
            nc.vector.memset(onesT[:], 1.0)
            lnw_t, lnb_t = [], []
            for li in range(3):
                t1 = constp.tile([128, FOUT_L[li] // 128], f32, tag=f"lnw{li}")
                nc.sync.dma_start(t1[:], lnw_d[li][:])
                t2 = constp.tile([128, FOUT_L[li] // 128], f32, tag=f"lnb{li}")
                nc.sync.dma_start(t2[:], lnb_d[li][:])
                lnw_t.append(t1)
                lnb_t.append(t2)
            # resident W1 rhs (small; needed per destination window in layer 1)
            rt1 = constp.tile([128, 2, 3584], bf16, tag="rt1")
            nc.sync.dma_start(
                rt1[:], rhs_d[0][:, 0:3584].rearrange("(k p) n -> p k n", p=128))

            yT = None  # produced by each layer's LN for the next layer

            for li in range(4):
                H, C, fin, fout = HEADS_L[li], C_L[li], FIN_L[li], FOUT_L[li]
                tcol = TCOL_L[li]
                kch = fin // 128
                acols = 2 * H

                brow = constp.tile([1, 3584], bf16, tag="brow")
                nc.sync.dma_start(brow[:, :fout], brow_d[li][:])

                if li > 0:
                    # ===== dense: h = y @ W^T (+ al columns)
                    shard = dram.tile([NPAD, tcol], bf16, tag=f"shard{li}")
                    glob = dram.tile([N_CORES * NPAD, tcol], bf16, addr_space="Shared", tag=f"glob{li}")
                    ashard = dram.tile([NPAD, 64], f32, tag=f"ashard{li}")
                    aglob = dram.tile([N_CORES * NPAD, 64], f32, addr_space="Shared", tag=f"aglob{li}")
                    fcs = []
                    o = 0
                    while o < fout + acols:
                        w_ = min(512, fout + acols - o)
                        fcs.append((o, w_))
                        o += w_
                    with tc.tile_pool(name=f"psA{li}", bufs=2, space="PSUM") as mmp:
                        for (fo, fw) in fcs:
                            kgrps = [(k0, min(7, kch - k0)) for k0 in range(0, kch, 7)]
                            rts = []
                            for (k0, kn) in kgrps:
                                rt = rhsp.tile([128, 7, 512], bf16, tag="rhs")
                                nc.sync.dma_start(
                                    rt[:, :kn, :fw],
                                    rhs_d[li][k0 * 128:(k0 + kn) * 128, fo:fo + fw]
                                    .rearrange("(k p) n -> p k n", p=128))
                                rts.append(rt)
                            for t in range(NT):
                                ps = mmp.tile([128, 512], f32, tag="mm")
                                for kc in range(kch):
                                    nc.tensor.matmul(
                                        ps[:, :fw],
                                        yT[:, kc, t * 128:(t + 1) * 128],
                                        rts[kc // 7][:, kc % 7, :fw],
                                        start=(kc == 0),
                                        stop=(kc == kch - 1))
                                hw = max(0, min(fw, fout - fo))
                                if hw > 0:
                                    st = stagep.tile([128, 512], bf16, tag="stg")
                                    if t % 2 == 0:
                                        nc.scalar.activation(st[:, :hw], ps[:, :hw], ACTF.Copy)
                                    else:
                                        nc.vector.tensor_copy(st[:, :hw], ps[:, :hw])
                                    nc.sync.dma_start(
                                        shard[t * 128:(t + 1) * 128, fo:fo + hw], st[:, :hw])
                                if hw < fw:
                                    a0 = fo + hw - fout
                                    sa = stagep.tile([128, 64], f32, tag="stga")
                                    nc.vector.tensor_copy(sa[:, :fw - hw], ps[:, hw:fw])
                                    nc.sync.dma_start(
                                        ashard[t * 128:(t + 1) * 128, a0:a0 + fw - hw],
                                        sa[:, :fw - hw])

                    # small al AllGather first, big table AllGather second
                    nc.gpsimd.collective_compute(
                        "AllGather", ALU.bypass, ins=[ashard[:]], outs=[aglob[:]],
                        replica_groups=RG)
                    nc.gpsimd.collective_compute(
                        "AllGather", ALU.bypass, ins=[shard[:]], outs=[glob[:]],
                        replica_groups=RG)
                    glob_bf = glob[:]
                    al_f32 = aglob[:]
                    al_step = 64
                    g_step = tcol
                else:
                    glob_bf = xtab_d[:]
                    al_f32 = xtab_d[:].bitcast(f32)[:, 128:192]
                    al_step = 192
                    g_step = 384

                # ===== edge phase
                edgeps = tc.tile_pool(name=f"psB{li}", bufs=1, space="PSUM")
                edgep = edgeps.__enter__()
                if li < 3:
                    zT = ytp.tile([128, 28, NPAD], bf16, tag="yt")
                    stats = miscp.tile([128, 2 * NW], f32, tag="stats")

                # ---- softmax pre-phase for all windows (overlaps the big AG)
                alpha_all = ephp.tile([128, nch, 8], bf16, tag="alpha", bufs=1)
                for w in range(NW):
                    c0, ncwW = coff[w], ncw[w]
                    ne = ncwW * 128
                    As = ephp.tile([128, ncmax, 64], f32, tag="as", bufs=1)
                    Ad = ephp.tile([128, ncmax, 64], f32, tag="ad", bufs=1)
                    nc.gpsimd.dma_gather(
                        As[:, :ncwW, :], al_f32,
                        idxs_t[:, c0:c0 + ncwW, :], ne, ne, 64, elem_step=al_step)
                    nc.gpsimd.dma_gather(
                        Ad[:, :ncwW, :], al_f32,
                        idxd_t[:, c0:c0 + ncwW, :], ne, ne, 64, elem_step=al_step)
                    maskw = mwp.tile([128, ncmax, 128], bf16, tag="mw")
                    nc.sync.dma_start(
                        maskw[:, :ncwW, :],
                        mask_d[:, c0 * 128:(c0 + ncwW) * 128].rearrange(
                            "p (c d) -> p c d", c=ncwW))
                    maskTw = mwp.tile([128, ncmax, 128], bf16, tag="mwT")
                    nc.sync.dma_start(
                        maskTw[:, :ncwW, :],
                        maskT_d[:, c0 * 128:(c0 + ncwW) * 128].rearrange(
                            "p (c d) -> p c d", c=ncwW))

                    ev = ephp.tile([128, ncmax, 8], f32, tag="ev")
                    nc.vector.tensor_tensor(
                        ev[:, :ncwW, :H], As[:, :ncwW, 0:H], Ad[:, :ncwW, H:2 * H], ALU.add)
                    nc.vector.scalar_tensor_tensor(
                        ev[:, :ncwW, :H], ev[:, :ncwW, :H], 0.2, ev[:, :ncwW, :H],
                        ALU.mult, ALU.max)
                    nc.vector.tensor_scalar_min(ev[:, :ncwW, :H], ev[:, :ncwW, :H], EXP_CLAMP)
                    wv = ephp.tile([128, ncmax, 8], bf16, tag="wv")
                    nc.scalar.activation(wv[:, :ncwW, :H], ev[:, :ncwW, :H], ACTF.Exp)

                    ps_den = edgep.tile([128, 8], f32, tag="sm")
                    for c in range(ncwW):
                        nc.tensor.matmul(
                            ps_den[:, :H], maskw[:, c, :], wv[:, c, :H],
                            start=(c == 0), stop=(c == ncwW - 1))
                    rden_f = ephp.tile([128, 8], f32, tag="rdenf")
                    nc.vector.tensor_scalar_max(rden_f[:, :H], ps_den[:, :H], DEN_TINY)
                    rden2 = ephp.tile([128, 8], f32, tag="rden2")
                    nc.vector.reciprocal(rden2[:, :H], rden_f[:, :H])
                    rden = ephp.tile([128, 8], bf16, tag="rden")
                    nc.vector.tensor_copy(rden[:, :H], rden2[:, :H])
                    for c in range(ncwW):
                        ps_exp = edgep.tile([128, 8], f32, tag="sm")
                        nc.tensor.matmul(
                            ps_exp[:, :H], maskTw[:, c, :], rden[:, :H],
                            start=True, stop=True)
                        nc.vector.tensor_tensor(
                            alpha_all[:, c0 + c, :H], wv[:, c, :H], ps_exp[:, :H], ALU.mult)

                # ---- gather + scatter phase
                for w in range(NW):
                    c0, ncwW = coff[w], ncw[w]
                    maskw = mwp.tile([128, ncmax, 128], bf16, tag="mw")
                    nc.sync.dma_start(
                        maskw[:, :ncwW, :],
                        mask_d[:, c0 * 128:(c0 + ncwW) * 128].rearrange(
                            "p (c d) -> p c d", c=ncwW))

                    if li == 0:
                        ps_agg = edgep.tile([128, 8, 256], f32, tag="out")
                    else:
                        ps_out = edgep.tile([128, fout], f32, tag="out")

                    for cp in range(0, ncwW, 2):
                        cw = min(2, ncwW - cp)
                        G = gathp.tile([128, 2, tcol], bf16, tag="G")
                        nc.gpsimd.dma_gather(
                            G[:, :cw, :], glob_bf[:, 0:tcol],
                            idxs_t[:, c0 + cp:c0 + cp + cw, :], cw * 128, cw * 128,
                            tcol, elem_step=g_step)
                        for c in range(cp, cp + cw):
                            lhs = ephp.tile([128, 8, 128], bf16, tag="lhs")
                            nc.vector.tensor_tensor(
                                lhs[:, :H, :],
                                maskw[:, c, :].unsqueeze(1).broadcast_to([128, H, 128]),
                                alpha_all[:, c0 + c, :H].unsqueeze(2).broadcast_to([128, H, 128]),
                                ALU.mult)
                            if li == 0:
                                for h in range(H):
                                    nc.tensor.matmul(
                                        ps_agg[:, h, :], lhs[:, h, :], G[:, c - cp, :256],
                                        start=(c == 0 and h % 2 == 0), stop=False)
                            else:
                                o = 0
                                while o < fout:
                                    h = o // C
                                    e = min((h + 1) * C, (o // 512 + 1) * 512, fout)
                                    nc.tensor.matmul(
                                        ps_out[:, o:e], lhs[:, h, :], G[:, c - cp, o:e],
                                        start=(c == 0 and o % 512 == 0), stop=False)
                                    o = e

                    if li == 0:
                        # evict alpha-weighted x sums, transpose, apply W1 per head
                        xs = zp.tile([128, 8, 256], bf16, tag="xs", bufs=1)
                        nc.vector.tensor_copy(xs[:], ps_agg[:])
                        ps_out = edgep.tile([128, fout], f32, tag="out")
                        for h in range(H):
                            ps_tr = edgep.tile([128, 2, 128], bf16, tag="sm")
                            for kc in range(2):
                                nc.tensor.matmul(
                                    ps_tr[:, kc, :], xs[:, h, kc * 128:(kc + 1) * 128],
                                    ident_t[:], is_transpose=True,
                                    start=(kc == 0), stop=(kc == 1))
                            xaT = zp.tile([128, 2, 128], bf16, tag="xaT")
                            nc.vector.tensor_copy(xaT[:], ps_tr[:])
                            o = h * C
                            while o < (h + 1) * C:
                                e = min((o // 512 + 1) * 512, (h + 1) * C)
                                for kc in range(2):
                                    nc.tensor.matmul(
                                        ps_out[:, o:e], xaT[:, kc, :],
                                        rt1[:, kc, o:e],
                                        start=(kc == 0 and o % 512 == 0), stop=False)
                                o = e
                    # bias add
                    o = 0
                    while o < fout:
                        e = min(o + 512, fout)
                        nc.tensor.matmul(
                            ps_out[:, o:e], ones_t[:, w, :], brow[:, o:e],
                            start=False, stop=(e == fout))
                        o = e

                    if li < 3:
                        z = zp.tile([128, 3584], bf16, tag="z")
                        nc.scalar.activation(
                            z[:, :fout], ps_out[:, :fout], ACTF.Relu,
                            accum_out=stats[:, w:w + 1])
                        sq = zp.tile([128, 3584], bf16, tag="z")
                        nc.vector.scalar_tensor_tensor(
                            sq[:, :fout], z[:, :fout], 1.0, z[:, :fout],
                            ALU.mult, ALU.mult,
                            accum_out=stats[:, NW + w:NW + w + 1])
                        for q in range(0, fout // 128, 4):
                            qn = min(4, fout // 128 - q)
                            ps_t = edgep.tile([128, 4, 128], bf16, tag="sm")
                            for j in range(qn):
                                nc.tensor.matmul(
                                    ps_t[:, j, :], z[:, (q + j) * 128:(q + j + 1) * 128],
                                    ident_t[:], is_transpose=True,
                                    start=(j == 0), stop=(j == qn - 1))
                            nc.vector.tensor_copy(
                                zT[:, q:q + qn, w * 128:(w + 1) * 128],
                                ps_t[:, :qn, :])
                    else:
                        zf = zp.tile([128, 1024], f32, tag="z")
                        nc.scalar.activation(zf[:, :fout], ps_out[:, :fout], ACTF.Copy)
                        rows = min(128, NPC - w * 128)
                        nc.sync.dma_start(out_d[w * 128:w * 128 + rows, :], zf[:rows, :fout])

                # ===== graph LayerNorm + next yT
                if li < 3:
                    sdram = dram.tile([1, 64], f32, tag=f"sd{li}")
                    sglob = dram.tile([1, 64], f32, addr_space="Shared", tag=f"sg{li}")
                    ps_s = edgep.tile([1, 2 * NW], f32, tag="sm")
                    nc.tensor.matmul(ps_s[:], ones128[:], stats[:], start=True, stop=True)
                    ssum = miscp.tile([1, 4], f32, tag="ssum")
                    nc.vector.tensor_reduce(ssum[:, 0:1], ps_s[:, 0:NW], AX.X, ALU.add)
                    nc.vector.tensor_reduce(ssum[:, 1:2], ps_s[:, NW:2 * NW], AX.X, ALU.add)
                    nc.sync.dma_start(sdram[:, 0:2], ssum[:, 0:2])
                    nc.gpsimd.collective_compute(
                        "AllReduce", ALU.add, ins=[sdram[:]], outs=[sglob[:]],
                        replica_groups=RG)
                    gsum = miscp.tile([1, 8], f32, tag="gsum")
                    nc.sync.dma_start(gsum[:, 0:2], sglob[:, 0:2])
                    sc = miscp.tile([1, 8], f32, tag="sc")
                    inv_cnt = 1.0 / (float(N_NODES) * fout)
                    nc.vector.tensor_scalar_mul(sc[:, 0:2], gsum[:, 0:2], inv_cnt)
                    nc.vector.tensor_tensor(sc[:, 2:3], sc[:, 0:1], sc[:, 0:1], ALU.mult)
                    nc.vector.tensor_sub(sc[:, 3:4], sc[:, 1:2], sc[:, 2:3])
                    nc.vector.tensor_scalar_add(sc[:, 3:4], sc[:, 3:4], 1e-5)
                    nc.scalar.sqrt(sc[:, 4:5], sc[:, 3:4])
                    nc.vector.reciprocal(sc[:, 5:6], sc[:, 4:5])
                    mr = miscp.tile([1, 2], f32, tag="mr")
                    nc.vector.tensor_copy(mr[:, 0:1], sc[:, 0:1])
                    nc.vector.tensor_copy(mr[:, 1:2], sc[:, 5:6])
                    ps_b = edgep.tile([128, 2], f32, tag="sm")
                    nc.tensor.matmul(ps_b[:], onesT[:], mr[:], start=True, stop=True)
                    br = miscp.tile([128, 2], f32, tag="br")
                    nc.vector.tensor_copy(br[:], ps_b[:])
                    nfc = fout // 128
                    scl = miscp.tile([128, 32], f32, tag="scl")
                    bia = miscp.tile([128, 32], f32, tag="bia")
                    nc.vector.tensor_scalar(
                        scl[:, :nfc], lnw_t[li][:, :], br[:, 1:2], None, ALU.mult)
                    nc.vector.tensor_scalar(
                        bia[:, :nfc], scl[:, :nfc], br[:, 0:1], None, ALU.mult)
                    nc.vector.tensor_tensor(
                        bia[:, :nfc], lnb_t[li][:, :], bia[:, :nfc], ALU.subtract)
                    for q in range(nfc):
                        nc.vector.scalar_tensor_tensor(
                            zT[:, q, :], zT[:, q, :], scl[:, q:q + 1],
                            bia[:, q:q + 1].broadcast_to([128, NPAD]),
                            ALU.mult, ALU.add)
                    yT = zT
                    if debug:
                        nc.sync.dma_start(
                            dbg_zt[li][:], zT[:].rearrange("p q n -> p (q n)"))
                        nc.sync.dma_start(dbg_st[li][:], sc[:])
                edgeps.__exit__(None, None, None)

    nc.compile()
    _CACHE[key] = nc
    return nc


# ---------------------------------------------------------------- entry point
def kernel(**inputs):
    _install_ntff_hook()
    from concourse.bass_utils import run_bass_kernel_spmd

    ncw, per_core = prep_edges(inputs["edge_index"])
    params = prep_params(inputs)

    in_maps = []
    for k in range(N_CORES):
        m = dict(params)
        m.update(per_core[k])
        in_maps.append(m)

    nc = build(ncw)
    res = run_bass_kernel_spmd(nc, in_maps, core_ids=list(range(N_CORES)), trace=False)
    out = np.concatenate([res.results[k]["out"] for k in range(N_CORES)], axis=0)
    return out.astype(np.float32)


# revision 13
# speedup vs baseline: 1.1023x; 1.1023x over previous
"""GATNet (4-layer GAT, 10000 nodes / 50000 edges + self-loops) on 8 Trainium2 NeuronCores.

Self-contained: builds per-core shards on the host (edge bucketing by destination,
one-hot scatter masks, gather index tables), compiles one SPMD Bass program, runs it
on cores 0-7 via run_bass_kernel_spmd, and reassembles the full [10000, 1000] output.

Structure per layer:
  dense h = y @ W^T (bf16, attention projections folded as extra rhs columns)
  -> small AllGather of the per-node attention scores (al) + big AllGather of h;
     the whole softmax pre-phase (score gathers, e-values, segment denominators
     via one-hot matmuls, reciprocals, alphas) runs under the big AllGather
  -> gather phase: dma_gather of source h rows; one-hot scatter matmuls with the
     per-edge alpha folded into the stationary operand; bias via K=1 matmul
  -> ReLU eviction with fused row sums, graph-LayerNorm stats via tiny AllReduce,
     PE-transpose into feature-major for the next layer's lhsT.
Layer 1 never materializes h: by linearity sum_e alpha_e * (x W)[src_e] =
(sum_e alpha_e x[src_e]) W, so it scatters raw x rows (256 wide) and applies W1
once per destination window; its attention scores are computed exactly on the host.
"""
import sys
import types

import numpy as np
import ml_dtypes

BF16 = ml_dtypes.bfloat16

N_NODES = 10000
N_CORES = 8
NPC = 1250
NPAD = 1280
NT = 10
NW = 10
HEADS_L = [8, 8, 8, 1]
C_L = [448, 384, 256, 1000]
FIN_L = [256, 3584, 3072, 2048]
FOUT_L = [3584, 3072, 2048, 1000]
TCOL_L = [256, 3072, 2048, 1024]    # bf16 columns of the gather table (L1: raw x)
EXP_CLAMP = 35.0
DEN_TINY = 1e-30


def _install_ntff_hook():
    if "antenv.axon_hooks" in sys.modules:
        return
    try:
        import antenv
        from trn_agent_boot.trn_boot import _ntff_profile_via_ctypes
    except ImportError:
        return
    mod = types.ModuleType("antenv.axon_hooks")
    state = {"hook": None}
    mod.set_axon_ntff_profile_hook = lambda h: state.__setitem__("hook", h)
    mod.get_axon_ntff_profile_hook = lambda: state["hook"]
    sys.modules["antenv.axon_hooks"] = mod
    antenv.axon_hooks = mod
    mod.set_axon_ntff_profile_hook(_ntff_profile_via_ctypes("/opt/axon/libaxon_pjrt.so"))


# ---------------------------------------------------------------- host prep
def _table_row(n):
    return NPAD * (n // NPC) + (n % NPC)


def _wrap16(idx_chunk):
    w = idx_chunk.reshape(8, 16).T
    return np.tile(w, (8, 1)).astype(np.int16)


def prep_edges(edge_index):
    src = np.asarray(edge_index[0], dtype=np.int64)
    dst = np.asarray(edge_index[1], dtype=np.int64)
    src = np.concatenate([src, np.arange(N_NODES, dtype=np.int64)])
    dst = np.concatenate([dst, np.arange(N_NODES, dtype=np.int64)])

    buckets = [[[] for _ in range(NW)] for _ in range(N_CORES)]
    core_of = dst // NPC
    win_of = (dst % NPC) // 128
    order = np.argsort(dst, kind="stable")
    for e in order:
        buckets[core_of[e]][win_of[e]].append(e)

    ncw = []
    for w in range(NW):
        mx = max(len(buckets[k][w]) for k in range(N_CORES))
        ncw.append(max(1, -(-mx // 128)))
    nch = sum(ncw)

    per_core = []
    for k in range(N_CORES):
        idx_s = np.zeros((128, nch * 8), np.int16)
        idx_d = np.zeros((128, nch * 8), np.int16)
        mask = np.zeros((128, nch, 128), np.float32)
        maskT = np.zeros((128, nch, 128), np.float32)
        esrc = np.zeros((nch, 128), np.int64)
        edst = np.zeros((nch, 128), np.int64)
        ereal = np.zeros((nch, 128), bool)
        c0 = 0
        for w in range(NW):
            edges = buckets[k][w]
            for c in range(ncw[w]):
                part = edges[c * 128:(c + 1) * 128]
                srows = np.zeros(128, np.int64)
                drows = np.zeros(128, np.int64)
                for i, e in enumerate(part):
                    srows[i] = _table_row(src[e])
                    drows[i] = _table_row(dst[e])
                    esrc[c0 + c, i] = src[e]
                    edst[c0 + c, i] = dst[e]
                    ereal[c0 + c, i] = True
                    d_local = (dst[e] % NPC) - 128 * w
                    mask[i, c0 + c, d_local] = 1.0
                    maskT[d_local, c0 + c, i] = 1.0
                idx_s[:, (c0 + c) * 8:(c0 + c + 1) * 8] = _wrap16(srows)
                idx_d[:, (c0 + c) * 8:(c0 + c + 1) * 8] = _wrap16(drows)
            c0 += ncw[w]
        per_core.append(dict(
            idxs=idx_s, idxd=idx_d,
            mask=mask.reshape(128, nch * 128).astype(BF16),
            maskT=maskT.reshape(128, nch * 128).astype(BF16),
            _esrc=esrc, _edst=edst, _ereal=ereal,
        ))
    return tuple(ncw), per_core


def prep_alpha1(per_core, ncw, al1):
    """Exact layer-1 softmax on the host: alpha[e, h] per (chunk, slot)."""
    nch = sum(ncw)
    als = al1[:, :8].astype(np.float64)
    ald = al1[:, 8:].astype(np.float64)
    out = []
    for pc in per_core:
        esrc, edst, ereal = pc["_esrc"], pc["_edst"], pc["_ereal"]
        e = als[esrc] + ald[edst]                      # [nch, 128, 8]
        e = np.maximum(e, 0.2 * e)
        wv = np.exp(np.minimum(e, EXP_CLAMP)) * ereal[:, :, None]
        den = np.zeros((NPC, 8))
        np.add.at(den, (edst % NPC).reshape(-1), wv.reshape(-1, 8))
        alpha = wv / np.maximum(den[(edst % NPC)], 1e-300)
        # device layout: [part=slot, nch*8]
        return_arr = np.ascontiguousarray(
            alpha.transpose(1, 0, 2).reshape(128, nch * 8)).astype(BF16)
        out.append(return_arr)
    return out


def prep_params(inputs):
    p = {}
    x64 = np.asarray(inputs["x"], np.float64)
    al1 = None
    for li in range(4):
        H, C, fin, fout = HEADS_L[li], C_L[li], FIN_L[li], FOUT_L[li]
        W = np.asarray(inputs[f"W{li+1}"], np.float32)
        a_src = np.asarray(inputs[f"a_src{li+1}"], np.float32)
        a_dst = np.asarray(inputs[f"a_dst{li+1}"], np.float32)
        a_blk_s = np.zeros((fout, H), np.float32)
        a_blk_d = np.zeros((fout, H), np.float32)
        for h in range(H):
            a_blk_s[h * C:(h + 1) * C, h] = a_src[h]
            a_blk_d[h * C:(h + 1) * C, h] = a_dst[h]
        rhs = np.concatenate([W.T, W.T @ a_blk_s, W.T @ a_blk_d], axis=1)
        p[f"rhs{li+1}"] = np.ascontiguousarray(rhs).astype(BF16)
        p[f"brow{li+1}"] = np.asarray(inputs[f"b{li+1}"], np.float32).reshape(1, fout).astype(BF16)
        if li < 3:
            lw = np.asarray(inputs[f"ln{li+1}_w"], np.float32)
            lb = np.asarray(inputs[f"ln{li+1}_b"], np.float32)
            p[f"lnw{li+1}"] = np.ascontiguousarray(lw.reshape(fout // 128, 128).T)
            p[f"lnb{li+1}"] = np.ascontiguousarray(lb.reshape(fout // 128, 128).T)
        if li == 0:
            al1 = np.concatenate(
                [x64 @ (W.T @ a_blk_s).astype(np.float64),
                 x64 @ (W.T @ a_blk_d).astype(np.float64)], axis=1).astype(np.float32)
    # packed x table, replicated: [10240 rows, 768 bytes] = 512B x bf16 + 256B f32 al
    xtab = np.zeros((N_CORES * NPAD, 768), np.uint8)
    rows = _table_row(np.arange(N_NODES))
    xb = np.asarray(inputs["x"], np.float32).astype(BF16)
    xtab[rows, :512] = xb.view(np.uint8)
    alpad = np.zeros((N_NODES, 64), np.float32)
    alpad[:, :16] = al1
    xtab[rows, 512:768] = alpad.view(np.uint8)
    p["xtab"] = xtab.view(BF16)
    p["_al1"] = al1
    p["ident"] = np.eye(128, dtype=BF16)
    ones_b = np.zeros((1, NW * 128), np.float32)
    ones_b[0, :NPC] = 1.0
    p["ones_b"] = ones_b.astype(BF16)
    return p


# ---------------------------------------------------------------- device build
_CACHE = {}


def build(ncw, debug=False):
    key = (tuple(ncw), debug)
    if key in _CACHE:
        return _CACHE[key]

    import concourse.bacc as bacc
    import concourse.mybir as mybir
    import concourse.tile as tile
    from concourse.library_config import mlp

    f32 = mybir.dt.float32
    bf16 = mybir.dt.bfloat16
    i16 = mybir.dt.int16
    AX = mybir.AxisListType
    ALU = mybir.AluOpType
    ACTF = mybir.ActivationFunctionType

    nch = sum(ncw)
    ncmax = max(ncw)
    coff = [0]
    for w in range(NW):
        coff.append(coff[-1] + ncw[w])
    nc = bacc.Bacc("TRN2")

    xtab_d = nc.declare_dram_parameter("xtab", [N_CORES * NPAD, 384], bf16, isOutput=False)
    rhs_d, brow_d, lnw_d, lnb_d = [], [], [], []
    for li in range(4):
        H, fout, fin = HEADS_L[li], FOUT_L[li], FIN_L[li]
        rhs_d.append(nc.declare_dram_parameter(f"rhs{li+1}", [fin, fout + 2 * H], bf16, isOutput=False))
        brow_d.append(nc.declare_dram_parameter(f"brow{li+1}", [1, fout], bf16, isOutput=False))
        if li < 3:
            lnw_d.append(nc.declare_dram_parameter(f"lnw{li+1}", [128, fout // 128], f32, isOutput=False))
            lnb_d.append(nc.declare_dram_parameter(f"lnb{li+1}", [128, fout // 128], f32, isOutput=False))
    alpha1_d = nc.declare_dram_parameter("alpha1", [128, nch * 8], bf16, isOutput=False)
    idxs_d = nc.declare_dram_parameter("idxs", [128, nch * 8], i16, isOutput=False)
    idxd_d = nc.declare_dram_parameter("idxd", [128, nch * 8], i16, isOutput=False)
    mask_d = nc.declare_dram_parameter("mask", [128, nch * 128], bf16, isOutput=False)
    maskT_d = nc.declare_dram_parameter("maskT", [128, nch * 128], bf16, isOutput=False)
    ones_d = nc.declare_dram_parameter("ones_b", [1, NW * 128], bf16, isOutput=False)
    ident_d = nc.declare_dram_parameter("ident", [128, 128], bf16, isOutput=False)
    out_d = nc.declare_dram_parameter("out", [NPC, 1000], f32, isOutput=True)
    dbg_zt, dbg_st = [], []
    if debug:
        for li in range(3):
            dbg_zt.append(nc.declare_dram_parameter(
                f"dbg_zt{li}", [128, 28 * NPAD], bf16, isOutput=True))
            dbg_st.append(nc.declare_dram_parameter(
                f"dbg_st{li}", [1, 8], f32, isOutput=True))

    RG = [list(range(N_CORES))]

    with tile.TileContext(nc) as tc:
        with (
            tc.tile_pool(name="const", bufs=1) as constp,
            tc.tile_pool(name="yt", bufs=1) as ytp,
            tc.tile_pool(name="rhs", bufs=5) as rhsp,
            tc.tile_pool(name="stage", bufs=2) as stagep,
            tc.tile_pool(name="gath", bufs=2) as gathp,
            tc.tile_pool(name="mw", bufs=2) as mwp,
            tc.tile_pool(name="eph", bufs=2) as ephp,
            tc.tile_pool(name="z", bufs=2) as zp,
            tc.tile_pool(name="misc", bufs=2) as miscp,
            tc.tile_pool(name="dram", bufs=1, space="DRAM") as dram,
        ):
            nc.gpsimd.load_library(mlp)

            idxs_t = constp.tile([128, nch, 8], i16, tag="idxs")
            nc.sync.dma_start(idxs_t[:], idxs_d[:].rearrange("p (c d) -> p c d", c=nch))
            idxd_t = constp.tile([128, nch, 8], i16, tag="idxd")
            nc.sync.dma_start(idxd_t[:], idxd_d[:].rearrange("p (c d) -> p c d", c=nch))
            ones_t = constp.tile([1, NW, 128], bf16, tag="onesb")
            nc.sync.dma_start(ones_t[:], ones_d[:].rearrange("p (w d) -> p w d", w=NW))
            ident_t = constp.tile([128, 128], bf16, tag="ident")
            nc.sync.dma_start(ident_t[:], ident_d[:])
            ones128 = constp.tile([128, 1], f32, tag="ones128")
            nc.vector.memset(ones128[:], 1.0)
            onesT = constp.tile([1, 128], f32, tag="onesT")
            nc.vector.memset(onesT[:], 1.0)
            lnw_t, lnb_t = [], []
            for li in range(3):
                t1 = constp.tile([128, FOUT_L[li] // 128], f32, tag=f"lnw{li}")
                nc.sync.dma_start(t1[:], lnw_d[li][:])
                t2 = constp.tile([128, FOUT_L[li] // 128], f32, tag=f"lnb{li}")
                nc.sync.dma_start(t2[:], lnb_d[li][:])
                lnw_t.append(t1)
                lnb_t.append(t2)
            alpha1_t = constp.tile([128, nch, 8], bf16, tag="alpha1")
            nc.sync.dma_start(alpha1_t[:], alpha1_d[:].rearrange("p (c d) -> p c d", c=nch))
            # resident W1 rhs (small; needed per destination window in layer 1)
            rt1 = constp.tile([128, 2, 3584], bf16, tag="rt1")
            nc.sync.dma_start(
                rt1[:], rhs_d[0][:, 0:3584].rearrange("(k p) n -> p k n", p=128))

            yT = None  # produced by each layer's LN for the next layer

            for li in range(4):
                H, C, fin, fout = HEADS_L[li], C_L[li], FIN_L[li], FOUT_L[li]
                tcol = TCOL_L[li]
                kch = fin // 128
                acols = 2 * H

                brow = constp.tile([1, 3584], bf16, tag="brow")
                nc.sync.dma_start(brow[:, :fout], brow_d[li][:])

                if li > 0:
                    # ===== dense: h = y @ W^T (+ al columns); al chunk FIRST so the
                    # small al AllGather + softmax pre-phase overlap the dense phase
                    shard = dram.tile([NPAD, tcol], bf16, tag=f"shard{li}")
                    glob = dram.tile([N_CORES * NPAD, tcol], bf16, addr_space="Shared", tag=f"glob{li}")
                    ashard = dram.tile([NPAD, 64], f32, tag=f"ashard{li}")
                    aglob = dram.tile([N_CORES * NPAD, 64], f32, addr_space="Shared", tag=f"aglob{li}")
                    fcs = [(fout, acols)]
                    o = 0
                    while o < fout:
                        w_ = min(512, fout - o)
                        fcs.append((o, w_))
                        o += w_
                    with tc.tile_pool(name=f"psA{li}", bufs=2, space="PSUM") as mmp:
                        for fci, (fo, fw) in enumerate(fcs):
                            kgrps = [(k0, min(7, kch - k0)) for k0 in range(0, kch, 7)]
                            rts = []
                            for (k0, kn) in kgrps:
                                rt = rhsp.tile([128, 7, 512], bf16, tag="rhs")
                                nc.sync.dma_start(
                                    rt[:, :kn, :fw],
                                    rhs_d[li][k0 * 128:(k0 + kn) * 128, fo:fo + fw]
                                    .rearrange("(k p) n -> p k n", p=128))
                                rts.append(rt)
                            for t in range(NT):
                                ps = mmp.tile([128, 512], f32, tag="mm")
                                for kc in range(kch):
                                    nc.tensor.matmul(
                                        ps[:, :fw],
                                        yT[:, kc, t * 128:(t + 1) * 128],
                                        rts[kc // 7][:, kc % 7, :fw],
                                        start=(kc == 0),
                                        stop=(kc == kch - 1))
                                hw = max(0, min(fw, fout - fo))
                                if hw > 0:
                                    st = stagep.tile([128, 512], bf16, tag="stg")
                                    if t % 2 == 0:
                                        nc.scalar.activation(st[:, :hw], ps[:, :hw], ACTF.Copy)
                                    else:
                                        nc.vector.tensor_copy(st[:, :hw], ps[:, :hw])
                                    nc.sync.dma_start(
                                        shard[t * 128:(t + 1) * 128, fo:fo + hw], st[:, :hw])
                                if hw < fw:
                                    a0 = fo + hw - fout
                                    sa = stagep.tile([128, 64], f32, tag="stga")
                                    nc.vector.tensor_copy(sa[:, :fw - hw], ps[:, hw:fw])
                                    nc.sync.dma_start(
                                        ashard[t * 128:(t + 1) * 128, a0:a0 + fw - hw],
                                        sa[:, :fw - hw])
                            if fci == 0:
                                nc.gpsimd.collective_compute(
                                    "AllGather", ALU.bypass, ins=[ashard[:]], outs=[aglob[:]],
                                    replica_groups=RG)

                        # ---- softmax pre-phase, overlapping the dense phase
                        al_f32 = aglob[:]
                        al_step = 64
                        alpha_all = ephp.tile([128, nch, 8], bf16, tag="alpha", bufs=1)
                        for w in range(NW):
                            c0, ncwW = coff[w], ncw[w]
                            ne = ncwW * 128
                            As = ephp.tile([128, ncmax, 64], f32, tag="as", bufs=1)
                            Ad = ephp.tile([128, ncmax, 64], f32, tag="ad", bufs=1)
                            nc.gpsimd.dma_gather(
                                As[:, :ncwW, :], al_f32,
                                idxs_t[:, c0:c0 + ncwW, :], ne, ne, 64, elem_step=al_step)
                            nc.gpsimd.dma_gather(
                                Ad[:, :ncwW, :], al_f32,
                                idxd_t[:, c0:c0 + ncwW, :], ne, ne, 64, elem_step=al_step)
                            maskw = mwp.tile([128, ncmax, 128], bf16, tag="mw")
                            nc.scalar.dma_start(
                                maskw[:, :ncwW, :],
                                mask_d[:, c0 * 128:(c0 + ncwW) * 128].rearrange(
                                    "p (c d) -> p c d", c=ncwW))
                            maskTw = mwp.tile([128, ncmax, 128], bf16, tag="mwT")
                            nc.scalar.dma_start(
                                maskTw[:, :ncwW, :],
                                maskT_d[:, c0 * 128:(c0 + ncwW) * 128].rearrange(
                                    "p (c d) -> p c d", c=ncwW))

                            ev = ephp.tile([128, ncmax, 8], f32, tag="ev")
                            nc.vector.tensor_tensor(
                                ev[:, :ncwW, :H], As[:, :ncwW, 0:H], Ad[:, :ncwW, H:2 * H], ALU.add)
                            nc.vector.scalar_tensor_tensor(
                                ev[:, :ncwW, :H], ev[:, :ncwW, :H], 0.2, ev[:, :ncwW, :H],
                                ALU.mult, ALU.max)
                            nc.vector.tensor_scalar_min(ev[:, :ncwW, :H], ev[:, :ncwW, :H], EXP_CLAMP)
                            wv = ephp.tile([128, ncmax, 8], bf16, tag="wv")
                            nc.scalar.activation(wv[:, :ncwW, :H], ev[:, :ncwW, :H], ACTF.Exp)

                            ps_den = mmp.tile([128, 8], f32, tag="smA")
                            for c in range(ncwW):
                                nc.tensor.matmul(
                                    ps_den[:, :H], maskw[:, c, :], wv[:, c, :H],
                                    start=(c == 0), stop=(c == ncwW - 1))
                            rden_f = ephp.tile([128, 8], f32, tag="rdenf")
                            nc.vector.tensor_scalar_max(rden_f[:, :H], ps_den[:, :H], DEN_TINY)
                            rden2 = ephp.tile([128, 8], f32, tag="rden2")
                            nc.vector.reciprocal(rden2[:, :H], rden_f[:, :H])
                            rden = ephp.tile([128, 8], bf16, tag="rden")
                            nc.vector.tensor_copy(rden[:, :H], rden2[:, :H])
                            for c in range(ncwW):
                                ps_exp = mmp.tile([128, 8], f32, tag="smA")
                                nc.tensor.matmul(
                                    ps_exp[:, :H], maskTw[:, c, :], rden[:, :H],
                                    start=True, stop=True)
                                nc.vector.tensor_tensor(
                                    alpha_all[:, c0 + c, :H], wv[:, c, :H], ps_exp[:, :H], ALU.mult)

                    # big table AllGather (emitted after pre-phase so the gpsimd
                    # queue isn't blocked behind its completion wait)
                    nc.gpsimd.collective_compute(
                        "AllGather", ALU.bypass, ins=[shard[:]], outs=[glob[:]],
                        replica_groups=RG)
                    glob_bf = glob[:]
                    g_step = tcol
                else:
                    glob_bf = xtab_d[:]
                    g_step = 384
                    alpha_all = alpha1_t

                # ===== edge phase
                edgeps = tc.tile_pool(name=f"psB{li}", bufs=1, space="PSUM")
                edgep = edgeps.__enter__()
                if li < 3:
                    zT = ytp.tile([128, 28, NPAD], bf16, tag="yt")
                    stats = miscp.tile([128, 2 * NW], f32, tag="stats")

                # ---- gather + scatter phase
                for w in range(NW):
                    c0, ncwW = coff[w], ncw[w]
                    maskw = mwp.tile([128, ncmax, 128], bf16, tag="mw")
                    nc.scalar.dma_start(
                        maskw[:, :ncwW, :],
                        mask_d[:, c0 * 128:(c0 + ncwW) * 128].rearrange(
                            "p (c d) -> p c d", c=ncwW))

                    if li == 0:
                        ps_agg = edgep.tile([128, 8, 256], f32, tag="out")
                    else:
                        ps_out = edgep.tile([128, fout], f32, tag="out")

                    for cp in range(0, ncwW, 2):
                        cw = min(2, ncwW - cp)
                        G = gathp.tile([128, 2, tcol], bf16, tag="G")
                        nc.gpsimd.dma_gather(
                            G[:, :cw, :], glob_bf[:, 0:tcol],
                            idxs_t[:, c0 + cp:c0 + cp + cw, :], cw * 128, cw * 128,
                            tcol, elem_step=g_step)
                        for c in range(cp, cp + cw):
                            lhs = ephp.tile([128, 8, 128], bf16, tag="lhs")
                            nc.vector.tensor_tensor(
                                lhs[:, :H, :],
                                maskw[:, c, :].unsqueeze(1).broadcast_to([128, H, 128]),
                                alpha_all[:, c0 + c, :H].unsqueeze(2).broadcast_to([128, H, 128]),
                                ALU.mult)
                            if li == 0:
                                for h in range(H):
                                    nc.tensor.matmul(
                                        ps_agg[:, h, :], lhs[:, h, :], G[:, c - cp, :256],
                                        start=(c == 0 and h % 2 == 0), stop=False)
                            else:
                                o = 0
                                while o < fout:
                                    h = o // C
                                    e = min((h + 1) * C, (o // 512 + 1) * 512, fout)
                                    nc.tensor.matmul(
                                        ps_out[:, o:e], lhs[:, h, :], G[:, c - cp, o:e],
                                        start=(c == 0 and o % 512 == 0), stop=False)
                                    o = e

                    if li == 0:
                        # evict alpha-weighted x sums, transpose, apply W1 per head
                        xs = zp.tile([128, 8, 256], bf16, tag="xs", bufs=1)
                        nc.vector.tensor_copy(xs[:], ps_agg[:])
                        ps_out = edgep.tile([128, fout], f32, tag="out")
                        for h in range(H):
                            ps_tr = edgep.tile([128, 2, 128], bf16, tag="sm")
                            for kc in range(2):
                                nc.tensor.matmul(
                                    ps_tr[:, kc, :], xs[:, h, kc * 128:(kc + 1) * 128],
                                    ident_t[:], is_transpose=True,
                                    start=(kc == 0), stop=(kc == 1))
                            xaT = zp.tile([128, 2, 128], bf16, tag="xaT")
                            nc.vector.tensor_copy(xaT[:], ps_tr[:])
                            o = h * C
                            while o < (h + 1) * C:
                                e = min((o // 512 + 1) * 512, (h + 1) * C)
                                for kc in range(2):
                                    nc.tensor.matmul(
                                        ps_out[:, o:e], xaT[:, kc, :],
                                        rt1[:, kc, o:e],
                                        start=(kc == 0 and o % 512 == 0), stop=False)
                                o = e
                    # bias add
                    o = 0
                    while o < fout:
                        e = min(o + 512, fout)
                        nc.tensor.matmul(
                            ps_out[:, o:e], ones_t[:, w, :], brow[:, o:e],
                            start=False, stop=(e == fout))
                        o = e

                    if li < 3:
                        z = zp.tile([128, 3584], bf16, tag="z")
                        nc.scalar.activation(
                            z[:, :fout], ps_out[:, :fout], ACTF.Relu,
                            accum_out=stats[:, w:w + 1])
                        sq = zp.tile([128, 3584], bf16, tag="z")
                        nc.vector.scalar_tensor_tensor(
                            sq[:, :fout], z[:, :fout], 1.0, z[:, :fout],
                            ALU.mult, ALU.mult,
                            accum_out=stats[:, NW + w:NW + w + 1])
                        for q in range(0, fout // 128, 4):
                            qn = min(4, fout // 128 - q)
                            ps_t = edgep.tile([128, 4, 128], bf16, tag="sm")
                            for j in range(qn):
                                nc.tensor.matmul(
                                    ps_t[:, j, :], z[:, (q + j) * 128:(q + j + 1) * 128],
                                    ident_t[:], is_transpose=True,
                                    start=(j == 0), stop=(j == qn - 1))
                            nc.vector.tensor_copy(
                                zT[:, q:q + qn, w * 128:(w + 1) * 128],
                                ps_t[:, :qn, :])
                    else:
                        zf = zp.tile([128, 1024], f32, tag="z")
                        nc.scalar.activation(zf[:, :fout], ps_out[:, :fout], ACTF.Copy)
                        rows = min(128, NPC - w * 128)
                        nc.sync.dma_start(out_d[w * 128:w * 128 + rows, :], zf[:rows, :fout])

                # ===== graph LayerNorm + next yT
                if li < 3:
                    sdram = dram.tile([1, 64], f32, tag=f"sd{li}")
                    sglob = dram.tile([1, 64], f32, addr_space="Shared", tag=f"sg{li}")
                    ps_s = edgep.tile([1, 2 * NW], f32, tag="sm")
                    nc.tensor.matmul(ps_s[:], ones128[:], stats[:], start=True, stop=True)
                    ssum = miscp.tile([1, 4], f32, tag="ssum")
                    nc.vector.tensor_reduce(ssum[:, 0:1], ps_s[:, 0:NW], AX.X, ALU.add)
                    nc.vector.tensor_reduce(ssum[:, 1:2], ps_s[:, NW:2 * NW], AX.X, ALU.add)
                    nc.sync.dma_start(sdram[:, 0:2], ssum[:, 0:2])
                    nc.gpsimd.collective_compute(
                        "AllReduce", ALU.add, ins=[sdram[:]], outs=[sglob[:]],
                        replica_groups=RG)
                    gsum = miscp.tile([1, 8], f32, tag="gsum")
                    nc.sync.dma_start(gsum[:, 0:2], sglob[:, 0:2])
                    sc = miscp.tile([1, 8], f32, tag="sc")
                    inv_cnt = 1.0 / (float(N_NODES) * fout)
                    nc.vector.tensor_scalar_mul(sc[:, 0:2], gsum[:, 0:2], inv_cnt)
                    nc.vector.tensor_tensor(sc[:, 2:3], sc[:, 0:1], sc[:, 0:1], ALU.mult)
                    nc.vector.tensor_sub(sc[:, 3:4], sc[:, 1:2], sc[:, 2:3])
                    nc.vector.tensor_scalar_add(sc[:, 3:4], sc[:, 3:4], 1e-5)
                    nc.scalar.sqrt(sc[:, 4:5], sc[:, 3:4])
                    nc.vector.reciprocal(sc[:, 5:6], sc[:, 4:5])
                    mr = miscp.tile([1, 2], f32, tag="mr")
                    nc.vector.tensor_copy(mr[:, 0:1], sc[:, 0:1])
                    nc.vector.tensor_copy(mr[:, 1:2], sc[:, 5:6])
                    ps_b = edgep.tile([128, 2], f32, tag="sm")
                    nc.tensor.matmul(ps_b[:], onesT[:], mr[:], start=True, stop=True)
                    br = miscp.tile([128, 2], f32, tag="br")
                    nc.vector.tensor_copy(br[:], ps_b[:])
                    nfc = fout // 128
                    scl = miscp.tile([128, 32], f32, tag="scl")
                    bia = miscp.tile([128, 32], f32, tag="bia")
                    nc.vector.tensor_scalar(
                        scl[:, :nfc], lnw_t[li][:, :], br[:, 1:2], None, ALU.mult)
                    nc.vector.tensor_scalar(
                        bia[:, :nfc], scl[:, :nfc], br[:, 0:1], None, ALU.mult)
                    nc.vector.tensor_tensor(
                        bia[:, :nfc], lnb_t[li][:, :], bia[:, :nfc], ALU.subtract)
                    for q in range(nfc):
                        nc.vector.scalar_tensor_tensor(
                            zT[:, q, :], zT[:, q, :], scl[:, q:q + 1],
                            bia[:, q:q + 1].broadcast_to([128, NPAD]),
                            ALU.mult, ALU.add)
                    yT = zT
                    if debug:
                        nc.sync.dma_start(
                            dbg_zt[li][:], zT[:].rearrange("p q n -> p (q n)"))
                        nc.sync.dma_start(dbg_st[li][:], sc[:])
                edgeps.__exit__(None, None, None)

    nc.compile()
    _CACHE[key] = nc
    return nc


# ---------------------------------------------------------------- entry point
def make_in_maps(inputs):
    ncw, per_core = prep_edges(inputs["edge_index"])
    params = prep_params(inputs)
    alpha1 = prep_alpha1(per_core, ncw, params.pop("_al1"))
    in_maps = []
    for k in range(N_CORES):
        m = dict(params)
        m.update({kk: vv for kk, vv in per_core[k].items() if not kk.startswith("_")})
        m["alpha1"] = alpha1[k]
        in_maps.append(m)
    return ncw, in_maps


def kernel(**inputs):
    _install_ntff_hook()
    from concourse.bass_utils import run_bass_kernel_spmd

    ncw, in_maps = make_in_maps(inputs)
    nc = build(ncw)
    res = run_bass_kernel_spmd(nc, in_maps, core_ids=list(range(N_CORES)), trace=False)
    out = np.concatenate([res.results[k]["out"] for k in range(N_CORES)], axis=0)
    return out.astype(np.float32)


# revision 15
# speedup vs baseline: 1.1876x; 1.0773x over previous
"""GATNet (4-layer GAT, 10000 nodes / 50000 edges + self-loops) on 8 Trainium2 NeuronCores.

Self-contained: builds per-core shards on the host (edge bucketing by destination,
one-hot scatter masks, gather index tables), compiles one SPMD Bass program, runs it
on cores 0-7 via run_bass_kernel_spmd, and reassembles the full [10000, 1000] output.

Structure per layer:
  dense h = y @ W^T (bf16, attention projections folded as extra rhs columns)
  -> small AllGather of the per-node attention scores (al) + big AllGather of h;
     the whole softmax pre-phase (score gathers, e-values, segment denominators
     via one-hot matmuls, reciprocals, alphas) runs under the big AllGather
  -> gather phase: dma_gather of source h rows; one-hot scatter matmuls with the
     per-edge alpha folded into the stationary operand; bias via K=1 matmul
  -> ReLU eviction with fused row sums, graph-LayerNorm stats via tiny AllReduce,
     PE-transpose into feature-major for the next layer's lhsT.
Layer 1 never materializes h: by linearity sum_e alpha_e * (x W)[src_e] =
(sum_e alpha_e x[src_e]) W, so it scatters raw x rows (256 wide) and applies W1
once per destination window; its attention scores are computed exactly on the host.
"""
import sys
import types

import numpy as np
import ml_dtypes

BF16 = ml_dtypes.bfloat16

N_NODES = 10000
N_CORES = 8
NPC = 1250
NPAD = 1280
NT = 10
NW = 10
HEADS_L = [8, 8, 8, 1]
C_L = [448, 384, 256, 1000]
FIN_L = [256, 3584, 3072, 2048]
FOUT_L = [3584, 3072, 2048, 1000]
TCOL_L = [256, 3072, 2048, 1024]    # bf16 columns of the gather table (L1: raw x)
EXP_CLAMP = 35.0
DEN_TINY = 1e-30


def _install_ntff_hook():
    if "antenv.axon_hooks" in sys.modules:
        return
    try:
        import antenv
        from trn_agent_boot.trn_boot import _ntff_profile_via_ctypes
    except ImportError:
        return
    mod = types.ModuleType("antenv.axon_hooks")
    state = {"hook": None}
    mod.set_axon_ntff_profile_hook = lambda h: state.__setitem__("hook", h)
    mod.get_axon_ntff_profile_hook = lambda: state["hook"]
    sys.modules["antenv.axon_hooks"] = mod
    antenv.axon_hooks = mod
    mod.set_axon_ntff_profile_hook(_ntff_profile_via_ctypes("/opt/axon/libaxon_pjrt.so"))


# ---------------------------------------------------------------- host prep
def _table_row(n):
    return NPAD * (n // NPC) + (n % NPC)


def _wrap16(idx_chunk):
    w = idx_chunk.reshape(8, 16).T
    return np.tile(w, (8, 1)).astype(np.int16)


def prep_edges(edge_index):
    src = np.asarray(edge_index[0], dtype=np.int64)
    dst = np.asarray(edge_index[1], dtype=np.int64)
    src = np.concatenate([src, np.arange(N_NODES, dtype=np.int64)])
    dst = np.concatenate([dst, np.arange(N_NODES, dtype=np.int64)])

    buckets = [[[] for _ in range(NW)] for _ in range(N_CORES)]
    core_of = dst // NPC
    win_of = (dst % NPC) // 128
    order = np.argsort(dst, kind="stable")
    for e in order:
        buckets[core_of[e]][win_of[e]].append(e)

    ncw = []
    for w in range(NW):
        mx = max(len(buckets[k][w]) for k in range(N_CORES))
        ncw.append(max(1, -(-mx // 128)))
    nch = sum(ncw)

    per_core = []
    for k in range(N_CORES):
        idx_s = np.zeros((128, nch * 8), np.int16)
        idx_d = np.zeros((128, nch * 8), np.int16)
        mask = np.zeros((128, nch, 128), np.float32)
        maskT = np.zeros((128, nch, 128), np.float32)
        esrc = np.zeros((nch, 128), np.int64)
        edst = np.zeros((nch, 128), np.int64)
        ereal = np.zeros((nch, 128), bool)
        c0 = 0
        for w in range(NW):
            edges = buckets[k][w]
            for c in range(ncw[w]):
                part = edges[c * 128:(c + 1) * 128]
                srows = np.zeros(128, np.int64)
                drows = np.zeros(128, np.int64)
                for i, e in enumerate(part):
                    srows[i] = _table_row(src[e])
                    drows[i] = _table_row(dst[e])
                    esrc[c0 + c, i] = src[e]
                    edst[c0 + c, i] = dst[e]
                    ereal[c0 + c, i] = True
                    d_local = (dst[e] % NPC) - 128 * w
                    mask[i, c0 + c, d_local] = 1.0
                    maskT[d_local, c0 + c, i] = 1.0
                idx_s[:, (c0 + c) * 8:(c0 + c + 1) * 8] = _wrap16(srows)
                idx_d[:, (c0 + c) * 8:(c0 + c + 1) * 8] = _wrap16(drows)
            c0 += ncw[w]
        per_core.append(dict(
            idxs=idx_s, idxd=idx_d,
            mask=mask.reshape(128, nch * 128).astype(BF16),
            maskT=maskT.reshape(128, nch * 128).astype(BF16),
            _esrc=esrc, _edst=edst, _ereal=ereal,
        ))
    return tuple(ncw), per_core


def prep_alpha1(per_core, ncw, al1):
    """Exact layer-1 softmax on the host: alpha[e, h] per (chunk, slot)."""
    nch = sum(ncw)
    als = al1[:, :8].astype(np.float64)
    ald = al1[:, 8:].astype(np.float64)
    out = []
    for pc in per_core:
        esrc, edst, ereal = pc["_esrc"], pc["_edst"], pc["_ereal"]
        e = als[esrc] + ald[edst]                      # [nch, 128, 8]
        e = np.maximum(e, 0.2 * e)
        wv = np.exp(np.minimum(e, EXP_CLAMP)) * ereal[:, :, None]
        den = np.zeros((NPC, 8))
        np.add.at(den, (edst % NPC).reshape(-1), wv.reshape(-1, 8))
        alpha = wv / np.maximum(den[(edst % NPC)], 1e-300)
        # device layout: [part=slot, nch*8]
        return_arr = np.ascontiguousarray(
            alpha.transpose(1, 0, 2).reshape(128, nch * 8)).astype(BF16)
        out.append(return_arr)
    return out


def prep_params(inputs):
    p = {}
    x64 = np.asarray(inputs["x"], np.float64)
    al1 = None
    for li in range(4):
        H, C, fin, fout = HEADS_L[li], C_L[li], FIN_L[li], FOUT_L[li]
        W = np.asarray(inputs[f"W{li+1}"], np.float32)
        a_src = np.asarray(inputs[f"a_src{li+1}"], np.float32)
        a_dst = np.asarray(inputs[f"a_dst{li+1}"], np.float32)
        a_blk_s = np.zeros((fout, H), np.float32)
        a_blk_d = np.zeros((fout, H), np.float32)
        for h in range(H):
            a_blk_s[h * C:(h + 1) * C, h] = a_src[h]
            a_blk_d[h * C:(h + 1) * C, h] = a_dst[h]
        rhs = np.concatenate([W.T, W.T @ a_blk_s, W.T @ a_blk_d], axis=1)
        p[f"rhs{li+1}"] = np.ascontiguousarray(rhs).astype(BF16)
        p[f"brow{li+1}"] = np.asarray(inputs[f"b{li+1}"], np.float32).reshape(1, fout).astype(BF16)
        if li < 3:
            lw = np.asarray(inputs[f"ln{li+1}_w"], np.float32)
            lb = np.asarray(inputs[f"ln{li+1}_b"], np.float32)
            p[f"lnw{li+1}"] = np.ascontiguousarray(lw.reshape(fout // 128, 128).T)
            p[f"lnb{li+1}"] = np.ascontiguousarray(lb.reshape(fout // 128, 128).T)
        if li == 0:
            al1 = np.concatenate(
                [x64 @ (W.T @ a_blk_s).astype(np.float64),
                 x64 @ (W.T @ a_blk_d).astype(np.float64)], axis=1).astype(np.float32)
    # packed x table, replicated: [10240 rows, 768 bytes] = 512B x bf16 + 256B f32 al
    xtab = np.zeros((N_CORES * NPAD, 768), np.uint8)
    rows = _table_row(np.arange(N_NODES))
    xb = np.asarray(inputs["x"], np.float32).astype(BF16)
    xtab[rows, :512] = xb.view(np.uint8)
    alpad = np.zeros((N_NODES, 64), np.float32)
    alpad[:, :16] = al1
    xtab[rows, 512:768] = alpad.view(np.uint8)
    p["xtab"] = xtab.view(BF16)
    p["_al1"] = al1
    p["ident"] = np.eye(128, dtype=BF16)
    ones_b = np.zeros((1, NW * 128), np.float32)
    ones_b[0, :NPC] = 1.0
    p["ones_b"] = ones_b.astype(BF16)
    return p


# ---------------------------------------------------------------- device build
_CACHE = {}


def build(ncw, debug=False):
    key = (tuple(ncw), debug)
    if key in _CACHE:
        return _CACHE[key]

    import concourse.bacc as bacc
    import concourse.mybir as mybir
    import concourse.tile as tile
    from concourse.library_config import mlp

    f32 = mybir.dt.float32
    bf16 = mybir.dt.bfloat16
    i16 = mybir.dt.int16
    AX = mybir.AxisListType
    ALU = mybir.AluOpType
    ACTF = mybir.ActivationFunctionType

    nch = sum(ncw)
    ncmax = max(ncw)
    coff = [0]
    for w in range(NW):
        coff.append(coff[-1] + ncw[w])
    nc = bacc.Bacc("TRN2", num_swdge_queues=4)

    xtab_d = nc.declare_dram_parameter("xtab", [N_CORES * NPAD, 384], bf16, isOutput=False)
    rhs_d, brow_d, lnw_d, lnb_d = [], [], [], []
    for li in range(4):
        H, fout, fin = HEADS_L[li], FOUT_L[li], FIN_L[li]
        rhs_d.append(nc.declare_dram_parameter(f"rhs{li+1}", [fin, fout + 2 * H], bf16, isOutput=False))
        brow_d.append(nc.declare_dram_parameter(f"brow{li+1}", [1, fout], bf16, isOutput=False))
        if li < 3:
            lnw_d.append(nc.declare_dram_parameter(f"lnw{li+1}", [128, fout // 128], f32, isOutput=False))
            lnb_d.append(nc.declare_dram_parameter(f"lnb{li+1}", [128, fout // 128], f32, isOutput=False))
    alpha1_d = nc.declare_dram_parameter("alpha1", [128, nch * 8], bf16, isOutput=False)
    idxs_d = nc.declare_dram_parameter("idxs", [128, nch * 8], i16, isOutput=False)
    idxd_d = nc.declare_dram_parameter("idxd", [128, nch * 8], i16, isOutput=False)
    mask_d = nc.declare_dram_parameter("mask", [128, nch * 128], bf16, isOutput=False)
    maskT_d = nc.declare_dram_parameter("maskT", [128, nch * 128], bf16, isOutput=False)
    ones_d = nc.declare_dram_parameter("ones_b", [1, NW * 128], bf16, isOutput=False)
    ident_d = nc.declare_dram_parameter("ident", [128, 128], bf16, isOutput=False)
    out_d = nc.declare_dram_parameter("out", [NPC, 1000], f32, isOutput=True)
    dbg_zt, dbg_st = [], []
    if debug:
        for li in range(3):
            dbg_zt.append(nc.declare_dram_parameter(
                f"dbg_zt{li}", [128, 28 * NPAD], bf16, isOutput=True))
            dbg_st.append(nc.declare_dram_parameter(
                f"dbg_st{li}", [1, 8], f32, isOutput=True))

    RG = [list(range(N_CORES))]

    with tile.TileContext(nc) as tc:
        with (
            tc.tile_pool(name="const", bufs=1) as constp,
            tc.tile_pool(name="yt", bufs=1) as ytp,
            tc.tile_pool(name="rhs", bufs=5) as rhsp,
            tc.tile_pool(name="stage", bufs=2) as stagep,
            tc.tile_pool(name="gath", bufs=2) as gathp,
            tc.tile_pool(name="mw", bufs=2) as mwp,
            tc.tile_pool(name="eph", bufs=2) as ephp,
            tc.tile_pool(name="z", bufs=2) as zp,
            tc.tile_pool(name="misc", bufs=2) as miscp,
            tc.tile_pool(name="dram", bufs=1, space="DRAM") as dram,
        ):
            nc.gpsimd.load_library(mlp)

            idxs_t = constp.tile([128, nch, 8], i16, tag="idxs")
            nc.sync.dma_start(idxs_t[:], idxs_d[:].rearrange("p (c d) -> p c d", c=nch))
            idxd_t = constp.tile([128, nch, 8], i16, tag="idxd")
            nc.sync.dma_start(idxd_t[:], idxd_d[:].rearrange("p (c d) -> p c d", c=nch))
            ones_t = constp.tile([1, NW, 128], bf16, tag="onesb")
            nc.sync.dma_start(ones_t[:], ones_d[:].rearrange("p (w d) -> p w d", w=NW))
            ident_t = constp.tile([128, 128], bf16, tag="ident")
            nc.sync.dma_start(ident_t[:], ident_d[:])
            ones128 = constp.tile([128, 1], f32, tag="ones128")
            nc.vector.memset(ones128[:], 1.0)
            onesT = constp.tile([1, 128], f32, tag="onesT")
            nc.vector.memset(onesT[:], 1.0)
            lnw_t, lnb_t = [], []
            for li in range(3):
                t1 = constp.tile([128, FOUT_L[li] // 128], f32, tag=f"lnw{li}")
                nc.sync.dma_start(t1[:], lnw_d[li][:])
                t2 = constp.tile([128, FOUT_L[li] // 128], f32, tag=f"lnb{li}")
                nc.sync.dma_start(t2[:], lnb_d[li][:])
                lnw_t.append(t1)
                lnb_t.append(t2)
            alpha1_t = constp.tile([128, nch, 8], bf16, tag="alpha1")
            nc.sync.dma_start(alpha1_t[:], alpha1_d[:].rearrange("p (c d) -> p c d", c=nch))
            # resident W1 rhs (small; needed per destination window in layer 1)
            rt1 = constp.tile([128, 2, 3584], bf16, tag="rt1")
            nc.sync.dma_start(
                rt1[:], rhs_d[0][:, 0:3584].rearrange("(k p) n -> p k n", p=128))

            yT = None  # produced by each layer's LN for the next layer

            for li in range(4):
                H, C, fin, fout = HEADS_L[li], C_L[li], FIN_L[li], FOUT_L[li]
                tcol = TCOL_L[li]
                kch = fin // 128
                acols = 2 * H

                brow = constp.tile([1, 3584], bf16, tag="brow")
                nc.sync.dma_start(brow[:, :fout], brow_d[li][:])

                if li > 0:
                    # ===== dense: h = y @ W^T (+ al columns); al chunk FIRST so the
                    # small al AllGather + softmax pre-phase overlap the dense phase
                    shard = dram.tile([NPAD, tcol], bf16, tag=f"shard{li}")
                    glob = dram.tile([N_CORES * NPAD, tcol], bf16, addr_space="Shared", tag=f"glob{li}")
                    ashard = dram.tile([NPAD, 64], f32, tag=f"ashard{li}")
                    aglob = dram.tile([N_CORES * NPAD, 64], f32, addr_space="Shared", tag=f"aglob{li}")
                    fcs = [(fout, acols)]
                    o = 0
                    while o < fout:
                        w_ = min(512, fout - o)
                        fcs.append((o, w_))
                        o += w_
                    with tc.tile_pool(name=f"psA{li}", bufs=2, space="PSUM") as mmp:
                        for fci, (fo, fw) in enumerate(fcs):
                            kgrps = [(k0, min(7, kch - k0)) for k0 in range(0, kch, 7)]
                            rts = []
                            for (k0, kn) in kgrps:
                                rt = rhsp.tile([128, 7, 512], bf16, tag="rhs")
                                nc.sync.dma_start(
                                    rt[:, :kn, :fw],
                                    rhs_d[li][k0 * 128:(k0 + kn) * 128, fo:fo + fw]
                                    .rearrange("(k p) n -> p k n", p=128))
                                rts.append(rt)
                            for t in range(NT):
                                ps = mmp.tile([128, 512], f32, tag="mm")
                                for kc in range(kch):
                                    nc.tensor.matmul(
                                        ps[:, :fw],
                                        yT[:, kc, t * 128:(t + 1) * 128],
                                        rts[kc // 7][:, kc % 7, :fw],
                                        start=(kc == 0),
                                        stop=(kc == kch - 1))
                                hw = max(0, min(fw, fout - fo))
                                if hw > 0:
                                    st = stagep.tile([128, 512], bf16, tag="stg")
                                    if t % 2 == 0:
                                        nc.scalar.activation(st[:, :hw], ps[:, :hw], ACTF.Copy)
                                    else:
                                        nc.vector.tensor_copy(st[:, :hw], ps[:, :hw])
                                    nc.sync.dma_start(
                                        shard[t * 128:(t + 1) * 128, fo:fo + hw], st[:, :hw])
                                if hw < fw:
                                    a0 = fo + hw - fout
                                    sa = stagep.tile([128, 64], f32, tag="stga")
                                    nc.vector.tensor_copy(sa[:, :fw - hw], ps[:, hw:fw])
                                    nc.sync.dma_start(
                                        ashard[t * 128:(t + 1) * 128, a0:a0 + fw - hw],
                                        sa[:, :fw - hw])
                            if fci == 0:
                                nc.gpsimd.collective_compute(
                                    "AllGather", ALU.bypass, ins=[ashard[:]], outs=[aglob[:]],
                                    replica_groups=RG)

                        # ---- softmax pre-phase, overlapping the dense phase
                        al_f32 = aglob[:]
                        al_step = 64
                        alpha_all = ephp.tile([128, nch, 8], bf16, tag="alpha", bufs=1)
                        for w in range(NW):
                            c0, ncwW = coff[w], ncw[w]
                            ne = ncwW * 128
                            As = ephp.tile([128, ncmax, 64], f32, tag="as", bufs=1)
                            Ad = ephp.tile([128, ncmax, 64], f32, tag="ad", bufs=1)
                            nc.gpsimd.dma_gather(
                                As[:, :ncwW, :], al_f32,
                                idxs_t[:, c0:c0 + ncwW, :], ne, ne, 64, elem_step=al_step,
                                queue_num=(2 * w) % 4)
                            nc.gpsimd.dma_gather(
                                Ad[:, :ncwW, :], al_f32,
                                idxd_t[:, c0:c0 + ncwW, :], ne, ne, 64, elem_step=al_step,
                                queue_num=(2 * w + 1) % 4)
                            maskw = mwp.tile([128, ncmax, 128], bf16, tag="mw")
                            nc.scalar.dma_start(
                                maskw[:, :ncwW, :],
                                mask_d[:, c0 * 128:(c0 + ncwW) * 128].rearrange(
                                    "p (c d) -> p c d", c=ncwW))
                            maskTw = mwp.tile([128, ncmax, 128], bf16, tag="mwT")
                            nc.scalar.dma_start(
                                maskTw[:, :ncwW, :],
                                maskT_d[:, c0 * 128:(c0 + ncwW) * 128].rearrange(
                                    "p (c d) -> p c d", c=ncwW))

                            ev = ephp.tile([128, ncmax, 8], f32, tag="ev")
                            nc.vector.tensor_tensor(
                                ev[:, :ncwW, :H], As[:, :ncwW, 0:H], Ad[:, :ncwW, H:2 * H], ALU.add)
                            nc.vector.scalar_tensor_tensor(
                                ev[:, :ncwW, :H], ev[:, :ncwW, :H], 0.2, ev[:, :ncwW, :H],
                                ALU.mult, ALU.max)
                            nc.vector.tensor_scalar_min(ev[:, :ncwW, :H], ev[:, :ncwW, :H], EXP_CLAMP)
                            wv = ephp.tile([128, ncmax, 8], bf16, tag="wv")
                            nc.scalar.activation(wv[:, :ncwW, :H], ev[:, :ncwW, :H], ACTF.Exp)

                            ps_den = mmp.tile([128, 8], f32, tag="smA")
                            for c in range(ncwW):
                                nc.tensor.matmul(
                                    ps_den[:, :H], maskw[:, c, :], wv[:, c, :H],
                                    start=(c == 0), stop=(c == ncwW - 1))
                            rden_f = ephp.tile([128, 8], f32, tag="rdenf")
                            nc.vector.tensor_scalar_max(rden_f[:, :H], ps_den[:, :H], DEN_TINY)
                            rden2 = ephp.tile([128, 8], f32, tag="rden2")
                            nc.vector.reciprocal(rden2[:, :H], rden_f[:, :H])
                            rden = ephp.tile([128, 8], bf16, tag="rden")
                            nc.vector.tensor_copy(rden[:, :H], rden2[:, :H])
                            for c in range(ncwW):
                                ps_exp = mmp.tile([128, 8], f32, tag="smA")
                                nc.tensor.matmul(
                                    ps_exp[:, :H], maskTw[:, c, :], rden[:, :H],
                                    start=True, stop=True)
                                nc.vector.tensor_tensor(
                                    alpha_all[:, c0 + c, :H], wv[:, c, :H], ps_exp[:, :H], ALU.mult)

                    # big table AllGather (emitted after pre-phase so the gpsimd
                    # queue isn't blocked behind its completion wait)
                    nc.gpsimd.collective_compute(
                        "AllGather", ALU.bypass, ins=[shard[:]], outs=[glob[:]],
                        replica_groups=RG)
                    glob_bf = glob[:]
                    g_step = tcol
                else:
                    glob_bf = xtab_d[:]
                    g_step = 384
                    alpha_all = alpha1_t

                # ===== edge phase
                edgeps = tc.tile_pool(name=f"psB{li}", bufs=1, space="PSUM")
                edgep = edgeps.__enter__()
                if li < 3:
                    zT = ytp.tile([128, 28, NPAD], bf16, tag="yt")
                    stats = miscp.tile([128, 2 * NW], f32, tag="stats")

                # ---- gather + scatter phase
                for w in range(NW):
                    c0, ncwW = coff[w], ncw[w]
                    maskw = mwp.tile([128, ncmax, 128], bf16, tag="mw")
                    nc.scalar.dma_start(
                        maskw[:, :ncwW, :],
                        mask_d[:, c0 * 128:(c0 + ncwW) * 128].rearrange(
                            "p (c d) -> p c d", c=ncwW))

                    if li == 0:
                        ps_agg = edgep.tile([128, 16, 128], f32, tag="out")
                    else:
                        ps_out = edgep.tile([128, fout], f32, tag="out")

                    for cp in range(0, ncwW, 2):
                        cw = min(2, ncwW - cp)
                        G = gathp.tile([128, 2, tcol], bf16, tag="G")
                        nc.gpsimd.dma_gather(
                            G[:, :cw, :], glob_bf[:, 0:tcol],
                            idxs_t[:, c0 + cp:c0 + cp + cw, :], cw * 128, cw * 128,
                            tcol, elem_step=g_step, queue_num=(cp // 2) % 4)
                        for c in range(cp, cp + cw):
                            lhs = ephp.tile([128, 8, 128], bf16, tag="lhs")
                            nc.vector.tensor_tensor(
                                lhs[:, :H, :],
                                maskw[:, c, :].unsqueeze(1).broadcast_to([128, H, 128]),
                                alpha_all[:, c0 + c, :H].unsqueeze(2).broadcast_to([128, H, 128]),
                                ALU.mult)
                            if li == 0:
                                for h in range(H):
                                    for kc in range(2):
                                        nc.tensor.matmul(
                                            ps_agg[:, 2 * h + kc, :],
                                            G[:, c - cp, kc * 128:(kc + 1) * 128],
                                            lhs[:, h, :],
                                            start=(c == 0 and (2 * h + kc) % 4 == 0),
                                            stop=False)
                            else:
                                o = 0
                                while o < fout:
                                    h = o // C
                                    e = min((h + 1) * C, (o // 512 + 1) * 512, fout)
                                    nc.tensor.matmul(
                                        ps_out[:, o:e], lhs[:, h, :], G[:, c - cp, o:e],
                                        start=(c == 0 and o % 512 == 0), stop=False)
                                    o = e

                    if li == 0:
                        # xaggT came out of the scatter matmuls already transposed
                        xs = zp.tile([128, 16, 128], bf16, tag="xs", bufs=1)
                        nc.vector.tensor_copy(xs[:], ps_agg[:])
                        ps_out = edgep.tile([128, fout], f32, tag="out")
                        for h in range(H):
                            o = h * C
                            while o < (h + 1) * C:
                                e = min((o // 512 + 1) * 512, (h + 1) * C)
                                for kc in range(2):
                                    nc.tensor.matmul(
                                        ps_out[:, o:e], xs[:, 2 * h + kc, :],
                                        rt1[:, kc, o:e],
                                        start=(kc == 0 and o % 512 == 0), stop=False)
                                o = e
                    # bias add
                    o = 0
                    while o < fout:
                        e = min(o + 512, fout)
                        nc.tensor.matmul(
                            ps_out[:, o:e], ones_t[:, w, :], brow[:, o:e],
                            start=False, stop=(e == fout))
                        o = e

                    if li < 3:
                        z = zp.tile([128, 3584], bf16, tag="z")
                        nc.scalar.activation(
                            z[:, :fout], ps_out[:, :fout], ACTF.Relu,
                            accum_out=stats[:, w:w + 1])
                        sq = zp.tile([128, 3584], bf16, tag="z")
                        nc.vector.scalar_tensor_tensor(
                            sq[:, :fout], z[:, :fout], 1.0, z[:, :fout],
                            ALU.mult, ALU.mult,
                            accum_out=stats[:, NW + w:NW + w + 1])
                        for q in range(0, fout // 128, 4):
                            qn = min(4, fout // 128 - q)
                            ps_t = edgep.tile([128, 4, 128], bf16, tag="sm")
                            for j in range(qn):
                                nc.tensor.matmul(
                                    ps_t[:, j, :], z[:, (q + j) * 128:(q + j + 1) * 128],
                                    ident_t[:], is_transpose=True,
                                    start=(j == 0), stop=(j == qn - 1))
                            nc.vector.tensor_copy(
                                zT[:, q:q + qn, w * 128:(w + 1) * 128],
                                ps_t[:, :qn, :])
                    else:
                        zf = zp.tile([128, 1024], f32, tag="z")
                        nc.scalar.activation(zf[:, :fout], ps_out[:, :fout], ACTF.Copy)
                        rows = min(128, NPC - w * 128)
                        nc.sync.dma_start(out_d[w * 128:w * 128 + rows, :], zf[:rows, :fout])

                # ===== graph LayerNorm + next yT
                if li < 3:
                    sdram = dram.tile([1, 64], f32, tag=f"sd{li}")
                    sglob = dram.tile([1, 64], f32, addr_space="Shared", tag=f"sg{li}")
                    ps_s = edgep.tile([1, 2 * NW], f32, tag="sm")
                    nc.tensor.matmul(ps_s[:], ones128[:], stats[:], start=True, stop=True)
                    ssum = miscp.tile([1, 4], f32, tag="ssum")
                    nc.vector.tensor_reduce(ssum[:, 0:1], ps_s[:, 0:NW], AX.X, ALU.add)
                    nc.vector.tensor_reduce(ssum[:, 1:2], ps_s[:, NW:2 * NW], AX.X, ALU.add)
                    nc.sync.dma_start(sdram[:, 0:2], ssum[:, 0:2])
                    nc.gpsimd.collective_compute(
                        "AllReduce", ALU.add, ins=[sdram[:]], outs=[sglob[:]],
                        replica_groups=RG)
                    gsum = miscp.tile([1, 8], f32, tag="gsum")
                    nc.sync.dma_start(gsum[:, 0:2], sglob[:, 0:2])
                    sc = miscp.tile([1, 8], f32, tag="sc")
                    inv_cnt = 1.0 / (float(N_NODES) * fout)
                    nc.vector.tensor_scalar_mul(sc[:, 0:2], gsum[:, 0:2], inv_cnt)
                    nc.vector.tensor_tensor(sc[:, 2:3], sc[:, 0:1], sc[:, 0:1], ALU.mult)
                    nc.vector.tensor_sub(sc[:, 3:4], sc[:, 1:2], sc[:, 2:3])
                    nc.vector.tensor_scalar_add(sc[:, 3:4], sc[:, 3:4], 1e-5)
                    nc.scalar.sqrt(sc[:, 4:5], sc[:, 3:4])
                    nc.vector.reciprocal(sc[:, 5:6], sc[:, 4:5])
                    mr = miscp.tile([1, 2], f32, tag="mr")
                    nc.vector.tensor_copy(mr[:, 0:1], sc[:, 0:1])
                    nc.vector.tensor_copy(mr[:, 1:2], sc[:, 5:6])
                    ps_b = edgep.tile([128, 2], f32, tag="sm")
                    nc.tensor.matmul(ps_b[:], onesT[:], mr[:], start=True, stop=True)
                    br = miscp.tile([128, 2], f32, tag="br")
                    nc.vector.tensor_copy(br[:], ps_b[:])
                    nfc = fout // 128
                    scl = miscp.tile([128, 32], f32, tag="scl")
                    bia = miscp.tile([128, 32], f32, tag="bia")
                    nc.vector.tensor_scalar(
                        scl[:, :nfc], lnw_t[li][:, :], br[:, 1:2], None, ALU.mult)
                    nc.vector.tensor_scalar(
                        bia[:, :nfc], scl[:, :nfc], br[:, 0:1], None, ALU.mult)
                    nc.vector.tensor_tensor(
                        bia[:, :nfc], lnb_t[li][:, :], bia[:, :nfc], ALU.subtract)
                    for q in range(nfc):
                        nc.vector.scalar_tensor_tensor(
                            zT[:, q, :], zT[:, q, :], scl[:, q:q + 1],
                            bia[:, q:q + 1].broadcast_to([128, NPAD]),
                            ALU.mult, ALU.add)
                    yT = zT
                    if debug:
                        nc.sync.dma_start(
                            dbg_zt[li][:], zT[:].rearrange("p q n -> p (q n)"))
                        nc.sync.dma_start(dbg_st[li][:], sc[:])
                edgeps.__exit__(None, None, None)

    nc.compile()
    _CACHE[key] = nc
    return nc


# ---------------------------------------------------------------- entry point
def make_in_maps(inputs):
    ncw, per_core = prep_edges(inputs["edge_index"])
    params = prep_params(inputs)
    alpha1 = prep_alpha1(per_core, ncw, params.pop("_al1"))
    in_maps = []
    for k in range(N_CORES):
        m = dict(params)
        m.update({kk: vv for kk, vv in per_core[k].items() if not kk.startswith("_")})
        m["alpha1"] = alpha1[k]
        in_maps.append(m)
    return ncw, in_maps


def kernel(**inputs):
    _install_ntff_hook()
    from concourse.bass_utils import run_bass_kernel_spmd

    ncw, in_maps = make_in_maps(inputs)
    nc = build(ncw)
    res = run_bass_kernel_spmd(nc, in_maps, core_ids=list(range(N_CORES)), trace=False)
    out = np.concatenate([res.results[k]["out"] for k in range(N_CORES)], axis=0)
    return out.astype(np.float32)


# revision 17
# speedup vs baseline: 1.2446x; 1.0480x over previous
"""GATNet (4-layer GAT, 10000 nodes / 50000 edges + self-loops) on 8 Trainium2 NeuronCores.

Self-contained: builds per-core shards on the host (edge bucketing by destination,
one-hot scatter masks, gather index tables), compiles one SPMD Bass program, runs it
on cores 0-7 via run_bass_kernel_spmd, and reassembles the full [10000, 1000] output.

Structure per layer:
  dense h = y @ W^T (bf16, attention projections folded as extra rhs columns)
  -> small AllGather of the per-node attention scores (al) + big AllGather of h;
     the whole softmax pre-phase (score gathers, e-values, segment denominators
     via one-hot matmuls, reciprocals, alphas) runs under the big AllGather
  -> gather phase: dma_gather of source h rows; one-hot scatter matmuls with the
     per-edge alpha folded into the stationary operand; bias via K=1 matmul
  -> ReLU eviction with fused row sums, graph-LayerNorm stats via tiny AllReduce,
     PE-transpose into feature-major for the next layer's lhsT.
Layer 1 never materializes h: by linearity sum_e alpha_e * (x W)[src_e] =
(sum_e alpha_e x[src_e]) W, so it scatters raw x rows (256 wide) and applies W1
once per destination window; its attention scores are computed exactly on the host.
"""
import sys
import types

import numpy as np
import ml_dtypes

BF16 = ml_dtypes.bfloat16

N_NODES = 10000
N_CORES = 8
NPC = 1250
NPAD = 1280
NT = 10
NW = 10
HEADS_L = [8, 8, 8, 1]
C_L = [448, 384, 256, 1000]
FIN_L = [256, 3584, 3072, 2048]
FOUT_L = [3584, 3072, 2048, 1000]
TCOL_L = [256, 3072, 2048, 1024]    # bf16 columns of the gather table (L1: raw x)
EXP_CLAMP = 35.0
DEN_TINY = 1e-30


def _install_ntff_hook():
    if "antenv.axon_hooks" in sys.modules:
        return
    try:
        import antenv
        from trn_agent_boot.trn_boot import _ntff_profile_via_ctypes
    except ImportError:
        return
    mod = types.ModuleType("antenv.axon_hooks")
    state = {"hook": None}
    mod.set_axon_ntff_profile_hook = lambda h: state.__setitem__("hook", h)
    mod.get_axon_ntff_profile_hook = lambda: state["hook"]
    sys.modules["antenv.axon_hooks"] = mod
    antenv.axon_hooks = mod
    mod.set_axon_ntff_profile_hook(_ntff_profile_via_ctypes("/opt/axon/libaxon_pjrt.so"))


# ---------------------------------------------------------------- host prep
def _table_row(n):
    return NPAD * (n // NPC) + (n % NPC)


def _wrap16(idx_chunk):
    w = idx_chunk.reshape(8, 16).T
    return np.tile(w, (8, 1)).astype(np.int16)


def prep_edges(edge_index):
    src = np.asarray(edge_index[0], dtype=np.int64)
    dst = np.asarray(edge_index[1], dtype=np.int64)
    src = np.concatenate([src, np.arange(N_NODES, dtype=np.int64)])
    dst = np.concatenate([dst, np.arange(N_NODES, dtype=np.int64)])

    buckets = [[[] for _ in range(NW)] for _ in range(N_CORES)]
    core_of = dst // NPC
    win_of = (dst % NPC) // 128
    order = np.argsort(dst, kind="stable")
    for e in order:
        buckets[core_of[e]][win_of[e]].append(e)

    ncw = []
    for w in range(NW):
        mx = max(len(buckets[k][w]) for k in range(N_CORES))
        ncw.append(max(1, -(-mx // 128)))
    nch = sum(ncw)

    per_core = []
    for k in range(N_CORES):
        idx_s = np.zeros((128, nch * 8), np.int16)
        idx_d = np.zeros((128, nch * 8), np.int16)
        mask = np.zeros((128, nch, 128), np.float32)
        maskT = np.zeros((128, nch, 128), np.float32)
        esrc = np.zeros((nch, 128), np.int64)
        edst = np.zeros((nch, 128), np.int64)
        ereal = np.zeros((nch, 128), bool)
        c0 = 0
        for w in range(NW):
            edges = buckets[k][w]
            for c in range(ncw[w]):
                part = edges[c * 128:(c + 1) * 128]
                srows = np.zeros(128, np.int64)
                drows = np.zeros(128, np.int64)
                for i, e in enumerate(part):
                    srows[i] = _table_row(src[e])
                    drows[i] = _table_row(dst[e])
                    esrc[c0 + c, i] = src[e]
                    edst[c0 + c, i] = dst[e]
                    ereal[c0 + c, i] = True
                    d_local = (dst[e] % NPC) - 128 * w
                    mask[i, c0 + c, d_local] = 1.0
                    maskT[d_local, c0 + c, i] = 1.0
                idx_s[:, (c0 + c) * 8:(c0 + c + 1) * 8] = _wrap16(srows)
                idx_d[:, (c0 + c) * 8:(c0 + c + 1) * 8] = _wrap16(drows)
            c0 += ncw[w]
        per_core.append(dict(
            idxs=idx_s, idxd=idx_d,
            mask=mask.reshape(128, nch * 128).astype(BF16),
            maskT=maskT.reshape(128, nch * 128).astype(BF16),
            _esrc=esrc, _edst=edst, _ereal=ereal,
        ))
    return tuple(ncw), per_core


def prep_alpha1(per_core, ncw, al1):
    """Exact layer-1 softmax on the host: alpha[e, h] per (chunk, slot)."""
    nch = sum(ncw)
    als = al1[:, :8].astype(np.float64)
    ald = al1[:, 8:].astype(np.float64)
    out = []
    for pc in per_core:
        esrc, edst, ereal = pc["_esrc"], pc["_edst"], pc["_ereal"]
        e = als[esrc] + ald[edst]                      # [nch, 128, 8]
        e = np.maximum(e, 0.2 * e)
        wv = np.exp(np.minimum(e, EXP_CLAMP)) * ereal[:, :, None]
        den = np.zeros((NPC, 8))
        np.add.at(den, (edst % NPC).reshape(-1), wv.reshape(-1, 8))
        alpha = wv / np.maximum(den[(edst % NPC)], 1e-300)
        # device layout: [part=slot, nch*8]
        return_arr = np.ascontiguousarray(
            alpha.transpose(1, 0, 2).reshape(128, nch * 8)).astype(BF16)
        out.append(return_arr)
    return out


def prep_params(inputs):
    p = {}
    x64 = np.asarray(inputs["x"], np.float64)
    al1 = None
    for li in range(4):
        H, C, fin, fout = HEADS_L[li], C_L[li], FIN_L[li], FOUT_L[li]
        W = np.asarray(inputs[f"W{li+1}"], np.float32)
        a_src = np.asarray(inputs[f"a_src{li+1}"], np.float32)
        a_dst = np.asarray(inputs[f"a_dst{li+1}"], np.float32)
        a_blk_s = np.zeros((fout, H), np.float32)
        a_blk_d = np.zeros((fout, H), np.float32)
        for h in range(H):
            a_blk_s[h * C:(h + 1) * C, h] = a_src[h]
            a_blk_d[h * C:(h + 1) * C, h] = a_dst[h]
        rhs = np.concatenate([W.T, W.T @ a_blk_s, W.T @ a_blk_d], axis=1)
        p[f"rhs{li+1}"] = np.ascontiguousarray(rhs).astype(BF16)
        p[f"brow{li+1}"] = np.asarray(inputs[f"b{li+1}"], np.float32).reshape(1, fout).astype(BF16)
        if li < 3:
            lw = np.asarray(inputs[f"ln{li+1}_w"], np.float32)
            lb = np.asarray(inputs[f"ln{li+1}_b"], np.float32)
            p[f"lnw{li+1}"] = np.ascontiguousarray(lw.reshape(fout // 128, 128).T)
            p[f"lnb{li+1}"] = np.ascontiguousarray(lb.reshape(fout // 128, 128).T)
        if li == 0:
            al1 = np.concatenate(
                [x64 @ (W.T @ a_blk_s).astype(np.float64),
                 x64 @ (W.T @ a_blk_d).astype(np.float64)], axis=1).astype(np.float32)
    # packed x table, replicated: [10240 rows, 768 bytes] = 512B x bf16 + 256B f32 al
    xtab = np.zeros((N_CORES * NPAD, 768), np.uint8)
    rows = _table_row(np.arange(N_NODES))
    xb = np.asarray(inputs["x"], np.float32).astype(BF16)
    xtab[rows, :512] = xb.view(np.uint8)
    alpad = np.zeros((N_NODES, 64), np.float32)
    alpad[:, :16] = al1
    xtab[rows, 512:768] = alpad.view(np.uint8)
    p["xtab"] = xtab.view(BF16)
    p["_al1"] = al1
    p["ident"] = np.eye(128, dtype=BF16)
    ones_b = np.zeros((1, NW * 128), np.float32)
    ones_b[0, :NPC] = 1.0
    p["ones_b"] = ones_b.astype(BF16)
    return p


# ---------------------------------------------------------------- device build
_CACHE = {}


def build(ncw, debug=False):
    key = (tuple(ncw), debug)
    if key in _CACHE:
        return _CACHE[key]

    import concourse.bacc as bacc
    import concourse.mybir as mybir
    import concourse.tile as tile
    from concourse.library_config import mlp

    f32 = mybir.dt.float32
    bf16 = mybir.dt.bfloat16
    i16 = mybir.dt.int16
    AX = mybir.AxisListType
    ALU = mybir.AluOpType
    ACTF = mybir.ActivationFunctionType

    nch = sum(ncw)
    ncmax = max(ncw)
    coff = [0]
    for w in range(NW):
        coff.append(coff[-1] + ncw[w])
    nc = bacc.Bacc("TRN2", num_swdge_queues=4)

    xtab_d = nc.declare_dram_parameter("xtab", [N_CORES * NPAD, 384], bf16, isOutput=False)
    rhs_d, brow_d, lnw_d, lnb_d = [], [], [], []
    for li in range(4):
        H, fout, fin = HEADS_L[li], FOUT_L[li], FIN_L[li]
        rhs_d.append(nc.declare_dram_parameter(f"rhs{li+1}", [fin, fout + 2 * H], bf16, isOutput=False))
        brow_d.append(nc.declare_dram_parameter(f"brow{li+1}", [1, fout], bf16, isOutput=False))
        if li < 3:
            lnw_d.append(nc.declare_dram_parameter(f"lnw{li+1}", [128, fout // 128], f32, isOutput=False))
            lnb_d.append(nc.declare_dram_parameter(f"lnb{li+1}", [128, fout // 128], f32, isOutput=False))
    alpha1_d = nc.declare_dram_parameter("alpha1", [128, nch * 8], bf16, isOutput=False)
    idxs_d = nc.declare_dram_parameter("idxs", [128, nch * 8], i16, isOutput=False)
    idxd_d = nc.declare_dram_parameter("idxd", [128, nch * 8], i16, isOutput=False)
    mask_d = nc.declare_dram_parameter("mask", [128, nch * 128], bf16, isOutput=False)
    maskT_d = nc.declare_dram_parameter("maskT", [128, nch * 128], bf16, isOutput=False)
    ones_d = nc.declare_dram_parameter("ones_b", [1, NW * 128], bf16, isOutput=False)
    ident_d = nc.declare_dram_parameter("ident", [128, 128], bf16, isOutput=False)
    out_d = nc.declare_dram_parameter("out", [NPC, 1000], f32, isOutput=True)
    dbg_zt, dbg_st = [], []
    if debug:
        for li in range(3):
            dbg_zt.append(nc.declare_dram_parameter(
                f"dbg_zt{li}", [128, 28 * NPAD], bf16, isOutput=True))
            dbg_st.append(nc.declare_dram_parameter(
                f"dbg_st{li}", [1, 8], f32, isOutput=True))

    RG = [list(range(N_CORES))]

    with tile.TileContext(nc) as tc:
        with (
            tc.tile_pool(name="const", bufs=1) as constp,
            tc.tile_pool(name="yt", bufs=1) as ytp,
            tc.tile_pool(name="rhs", bufs=5) as rhsp,
            tc.tile_pool(name="stage", bufs=2) as stagep,
            tc.tile_pool(name="gath", bufs=2) as gathp,
            tc.tile_pool(name="mw", bufs=2) as mwp,
            tc.tile_pool(name="eph", bufs=2) as ephp,
            tc.tile_pool(name="z", bufs=2) as zp,
            tc.tile_pool(name="misc", bufs=2) as miscp,
            tc.tile_pool(name="dram", bufs=1, space="DRAM") as dram,
        ):
            nc.gpsimd.load_library(mlp)

            idxs_t = constp.tile([128, nch, 8], i16, tag="idxs")
            nc.sync.dma_start(idxs_t[:], idxs_d[:].rearrange("p (c d) -> p c d", c=nch))
            idxd_t = constp.tile([128, nch, 8], i16, tag="idxd")
            nc.sync.dma_start(idxd_t[:], idxd_d[:].rearrange("p (c d) -> p c d", c=nch))
            ones_t = constp.tile([1, NW, 128], bf16, tag="onesb")
            nc.sync.dma_start(ones_t[:], ones_d[:].rearrange("p (w d) -> p w d", w=NW))
            ident_t = constp.tile([128, 128], bf16, tag="ident")
            nc.sync.dma_start(ident_t[:], ident_d[:])
            ones128 = constp.tile([128, 1], f32, tag="ones128")
            nc.vector.memset(ones128[:], 1.0)
            onesT = constp.tile([1, 128], f32, tag="onesT")
            nc.vector.memset(onesT[:], 1.0)
            lnw_t, lnb_t = [], []
            for li in range(3):
                t1 = constp.tile([128, FOUT_L[li] // 128], f32, tag=f"lnw{li}")
                nc.sync.dma_start(t1[:], lnw_d[li][:])
                t2 = constp.tile([128, FOUT_L[li] // 128], f32, tag=f"lnb{li}")
                nc.sync.dma_start(t2[:], lnb_d[li][:])
                lnw_t.append(t1)
                lnb_t.append(t2)
            alpha1_t = constp.tile([128, nch, 8], bf16, tag="alpha1")
            nc.sync.dma_start(alpha1_t[:], alpha1_d[:].rearrange("p (c d) -> p c d", c=nch))
            # resident W1 rhs (small; needed per destination window in layer 1)
            rt1 = constp.tile([128, 2, 3584], bf16, tag="rt1")
            nc.sync.dma_start(
                rt1[:], rhs_d[0][:, 0:3584].rearrange("(k p) n -> p k n", p=128))

            yT = None  # produced by each layer's LN for the next layer

            for li in range(4):
                H, C, fin, fout = HEADS_L[li], C_L[li], FIN_L[li], FOUT_L[li]
                tcol = TCOL_L[li]
                kch = fin // 128
                acols = 2 * H

                brow = constp.tile([1, 3584], bf16, tag="brow")
                nc.sync.dma_start(brow[:, :fout], brow_d[li][:])

                if li > 0:
                    # ===== dense: h = y @ W^T (+ al columns); al chunk FIRST so the
                    # small al AllGather + softmax pre-phase overlap the dense phase
                    half = {3072: 1536, 2048: 1024, 1024: 512}[tcol]
                    shard_a = dram.tile([NPAD, half], bf16, tag=f"sharda{li}")
                    shard_b = dram.tile([NPAD, tcol - half], bf16, tag=f"shardb{li}")
                    glob_a = dram.tile([N_CORES * NPAD, half], bf16, addr_space="Shared", tag=f"globa{li}")
                    glob_b = dram.tile([N_CORES * NPAD, tcol - half], bf16, addr_space="Shared", tag=f"globb{li}")
                    ashard = dram.tile([NPAD, 64], f32, tag=f"ashard{li}")
                    aglob = dram.tile([N_CORES * NPAD, 64], f32, addr_space="Shared", tag=f"aglob{li}")
                    fcs = [(fout, acols)]
                    o = 0
                    while o < fout:
                        w_ = min(512, fout - o)
                        fcs.append((o, w_))
                        o += w_
                    with tc.tile_pool(name=f"psA{li}", bufs=2, space="PSUM") as mmp:
                        for fci, (fo, fw) in enumerate(fcs):
                            kgrps = [(k0, min(7, kch - k0)) for k0 in range(0, kch, 7)]
                            rts = []
                            for (k0, kn) in kgrps:
                                rt = rhsp.tile([128, 7, 512], bf16, tag="rhs")
                                nc.sync.dma_start(
                                    rt[:, :kn, :fw],
                                    rhs_d[li][k0 * 128:(k0 + kn) * 128, fo:fo + fw]
                                    .rearrange("(k p) n -> p k n", p=128))
                                rts.append(rt)
                            for t in range(NT):
                                ps = mmp.tile([128, 512], f32, tag="mm")
                                for kc in range(kch):
                                    nc.tensor.matmul(
                                        ps[:, :fw],
                                        yT[:, kc, t * 128:(t + 1) * 128],
                                        rts[kc // 7][:, kc % 7, :fw],
                                        start=(kc == 0),
                                        stop=(kc == kch - 1))
                                hw = max(0, min(fw, fout - fo))
                                if hw > 0:
                                    st = stagep.tile([128, 512], bf16, tag="stg")
                                    if t % 2 == 0:
                                        nc.scalar.activation(st[:, :hw], ps[:, :hw], ACTF.Copy)
                                    else:
                                        nc.vector.tensor_copy(st[:, :hw], ps[:, :hw])
                                    if fo < half:
                                        nc.sync.dma_start(
                                            shard_a[t * 128:(t + 1) * 128, fo:fo + hw], st[:, :hw])
                                    else:
                                        nc.sync.dma_start(
                                            shard_b[t * 128:(t + 1) * 128, fo - half:fo - half + hw],
                                            st[:, :hw])
                                if hw < fw:
                                    a0 = fo + hw - fout
                                    sa = stagep.tile([128, 64], f32, tag="stga")
                                    nc.vector.tensor_copy(sa[:, :fw - hw], ps[:, hw:fw])
                                    nc.sync.dma_start(
                                        ashard[t * 128:(t + 1) * 128, a0:a0 + fw - hw],
                                        sa[:, :fw - hw])
                            if fci == 0:
                                nc.gpsimd.collective_compute(
                                    "AllGather", ALU.bypass, ins=[ashard[:]], outs=[aglob[:]],
                                    replica_groups=RG)
                            if fci == half // 512:
                                nc.gpsimd.collective_compute(
                                    "AllGather", ALU.bypass, ins=[shard_a[:]], outs=[glob_a[:]],
                                    replica_groups=RG)

                        # ---- softmax pre-phase, overlapping the dense phase
                        al_f32 = aglob[:]
                        al_step = 64
                        alpha_all = ephp.tile([128, nch, 8], bf16, tag="alpha", bufs=1)
                        for w in range(NW):
                            c0, ncwW = coff[w], ncw[w]
                            ne = ncwW * 128
                            As = ephp.tile([128, ncmax, 64], f32, tag="as", bufs=1)
                            Ad = ephp.tile([128, ncmax, 64], f32, tag="ad", bufs=1)
                            nc.gpsimd.dma_gather(
                                As[:, :ncwW, :], al_f32,
                                idxs_t[:, c0:c0 + ncwW, :], ne, ne, 64, elem_step=al_step,
                                queue_num=(2 * w) % 4)
                            nc.gpsimd.dma_gather(
                                Ad[:, :ncwW, :], al_f32,
                                idxd_t[:, c0:c0 + ncwW, :], ne, ne, 64, elem_step=al_step,
                                queue_num=(2 * w + 1) % 4)
                            maskw = mwp.tile([128, ncmax, 128], bf16, tag="mw")
                            nc.scalar.dma_start(
                                maskw[:, :ncwW, :],
                                mask_d[:, c0 * 128:(c0 + ncwW) * 128].rearrange(
                                    "p (c d) -> p c d", c=ncwW))
                            maskTw = mwp.tile([128, ncmax, 128], bf16, tag="mwT")
                            nc.scalar.dma_start(
                                maskTw[:, :ncwW, :],
                                maskT_d[:, c0 * 128:(c0 + ncwW) * 128].rearrange(
                                    "p (c d) -> p c d", c=ncwW))

                            ev = ephp.tile([128, ncmax, 8], f32, tag="ev")
                            nc.vector.tensor_tensor(
                                ev[:, :ncwW, :H], As[:, :ncwW, 0:H], Ad[:, :ncwW, H:2 * H], ALU.add)
                            nc.vector.scalar_tensor_tensor(
                                ev[:, :ncwW, :H], ev[:, :ncwW, :H], 0.2, ev[:, :ncwW, :H],
                                ALU.mult, ALU.max)
                            nc.vector.tensor_scalar_min(ev[:, :ncwW, :H], ev[:, :ncwW, :H], EXP_CLAMP)
                            wv = ephp.tile([128, ncmax, 8], bf16, tag="wv")
                            nc.scalar.activation(wv[:, :ncwW, :H], ev[:, :ncwW, :H], ACTF.Exp)

                            ps_den = mmp.tile([128, 8], f32, tag="smA")
                            for c in range(ncwW):
                                nc.tensor.matmul(
                                    ps_den[:, :H], maskw[:, c, :], wv[:, c, :H],
                                    start=(c == 0), stop=(c == ncwW - 1))
                            rden_f = ephp.tile([128, 8], f32, tag="rdenf")
                            nc.vector.tensor_scalar_max(rden_f[:, :H], ps_den[:, :H], DEN_TINY)
                            rden2 = ephp.tile([128, 8], f32, tag="rden2")
                            nc.vector.reciprocal(rden2[:, :H], rden_f[:, :H])
                            rden = ephp.tile([128, 8], bf16, tag="rden")
                            nc.vector.tensor_copy(rden[:, :H], rden2[:, :H])
                            for c in range(ncwW):
                                ps_exp = mmp.tile([128, 8], f32, tag="smA")
                                nc.tensor.matmul(
                                    ps_exp[:, :H], maskTw[:, c, :], rden[:, :H],
                                    start=True, stop=True)
                                nc.vector.tensor_tensor(
                                    alpha_all[:, c0 + c, :H], wv[:, c, :H], ps_exp[:, :H], ALU.mult)

                    # second-half table AllGather (emitted after pre-phase so the
                    # gpsimd queue isn't blocked behind its completion wait)
                    nc.gpsimd.collective_compute(
                        "AllGather", ALU.bypass, ins=[shard_b[:]], outs=[glob_b[:]],
                        replica_groups=RG)
                else:
                    alpha_all = alpha1_t

                # ===== edge phase
                edgeps = tc.tile_pool(name=f"psB{li}", bufs=1, space="PSUM")
                edgep = edgeps.__enter__()
                if li < 3:
                    zT = ytp.tile([128, 28, NPAD], bf16, tag="yt")
                    stats = miscp.tile([128, 2 * NW], f32, tag="stats")

                # ---- gather + scatter phase
                for w in range(NW):
                    c0, ncwW = coff[w], ncw[w]
                    maskw = mwp.tile([128, ncmax, 128], bf16, tag="mw")
                    nc.scalar.dma_start(
                        maskw[:, :ncwW, :],
                        mask_d[:, c0 * 128:(c0 + ncwW) * 128].rearrange(
                            "p (c d) -> p c d", c=ncwW))

                    if li == 0:
                        ps_agg = edgep.tile([128, 16, 128], f32, tag="out")
                    else:
                        ps_out = edgep.tile([128, fout], f32, tag="out")

                    for cp in range(0, ncwW, 2):
                        cw = min(2, ncwW - cp)
                        if li == 0:
                            G = gathp.tile([128, 2, 256], bf16, tag="G")
                            nc.gpsimd.dma_gather(
                                G[:, :cw, :], xtab_d[:][:, 0:256],
                                idxs_t[:, c0 + cp:c0 + cp + cw, :], cw * 128, cw * 128,
                                256, elem_step=384, queue_num=(cp // 2) % 4)
                        else:
                            G = gathp.tile([128, 2, half], bf16, tag="G")
                            nc.gpsimd.dma_gather(
                                G[:, :cw, :], glob_a[:],
                                idxs_t[:, c0 + cp:c0 + cp + cw, :], cw * 128, cw * 128,
                                half, elem_step=half, queue_num=(cp // 2) % 4)
                            Gb = gathp.tile([128, 2, tcol - half], bf16, tag="Gb")
                            nc.gpsimd.dma_gather(
                                Gb[:, :cw, :], glob_b[:],
                                idxs_t[:, c0 + cp:c0 + cp + cw, :], cw * 128, cw * 128,
                                tcol - half, elem_step=tcol - half,
                                queue_num=(cp // 2 + 2) % 4)
                        for c in range(cp, cp + cw):
                            lhs = ephp.tile([128, 8, 128], bf16, tag="lhs")
                            nc.vector.tensor_tensor(
                                lhs[:, :H, :],
                                maskw[:, c, :].unsqueeze(1).broadcast_to([128, H, 128]),
                                alpha_all[:, c0 + c, :H].unsqueeze(2).broadcast_to([128, H, 128]),
                                ALU.mult)
                            if li == 0:
                                for h in range(H):
                                    for kc in range(2):
                                        nc.tensor.matmul(
                                            ps_agg[:, 2 * h + kc, :],
                                            G[:, c - cp, kc * 128:(kc + 1) * 128],
                                            lhs[:, h, :],
                                            start=(c == 0 and (2 * h + kc) % 4 == 0),
                                            stop=False)
                            else:
                                o = 0
                                while o < fout:
                                    h = o // C
                                    e = min((h + 1) * C, (o // 512 + 1) * 512, fout)
                                    if o < half:
                                        rhs_g = G[:, c - cp, o:e]
                                    else:
                                        rhs_g = Gb[:, c - cp, o - half:e - half]
                                    nc.tensor.matmul(
                                        ps_out[:, o:e], lhs[:, h, :], rhs_g,
                                        start=(c == 0 and o % 512 == 0), stop=False)
                                    o = e

                    if li == 0:
                        # xaggT came out of the scatter matmuls already transposed
                        xs = zp.tile([128, 16, 128], bf16, tag="xs", bufs=1)
                        nc.vector.tensor_copy(xs[:], ps_agg[:])
                        ps_out = edgep.tile([128, fout], f32, tag="out")
                        for h in range(H):
                            o = h * C
                            while o < (h + 1) * C:
                                e = min((o // 512 + 1) * 512, (h + 1) * C)
                                for kc in range(2):
                                    nc.tensor.matmul(
                                        ps_out[:, o:e], xs[:, 2 * h + kc, :],
                                        rt1[:, kc, o:e],
                                        start=(kc == 0 and o % 512 == 0), stop=False)
                                o = e
                    # bias add
                    o = 0
                    while o < fout:
                        e = min(o + 512, fout)
                        nc.tensor.matmul(
                            ps_out[:, o:e], ones_t[:, w, :], brow[:, o:e],
                            start=False, stop=(e == fout))
                        o = e

                    if li < 3:
                        z = zp.tile([128, 3584], bf16, tag="z")
                        nc.scalar.activation(
                            z[:, :fout], ps_out[:, :fout], ACTF.Relu,
                            accum_out=stats[:, w:w + 1])
                        sq = zp.tile([128, 3584], bf16, tag="z")
                        nc.vector.scalar_tensor_tensor(
                            sq[:, :fout], z[:, :fout], 1.0, z[:, :fout],
                            ALU.mult, ALU.mult,
                            accum_out=stats[:, NW + w:NW + w + 1])
                        for q in range(0, fout // 128, 4):
                            qn = min(4, fout // 128 - q)
                            ps_t = edgep.tile([128, 4, 128], bf16, tag="sm")
                            for j in range(qn):
                                nc.tensor.matmul(
                                    ps_t[:, j, :], z[:, (q + j) * 128:(q + j + 1) * 128],
                                    ident_t[:], is_transpose=True,
                                    start=(j == 0), stop=(j == qn - 1))
                            nc.vector.tensor_copy(
                                zT[:, q:q + qn, w * 128:(w + 1) * 128],
                                ps_t[:, :qn, :])
                    else:
                        zf = zp.tile([128, 1024], f32, tag="z")
                        nc.scalar.activation(zf[:, :fout], ps_out[:, :fout], ACTF.Copy)
                        rows = min(128, NPC - w * 128)
                        nc.sync.dma_start(out_d[w * 128:w * 128 + rows, :], zf[:rows, :fout])

                # ===== graph LayerNorm + next yT
                if li < 3:
                    sdram = dram.tile([1, 64], f32, tag=f"sd{li}")
                    sglob = dram.tile([1, 64], f32, addr_space="Shared", tag=f"sg{li}")
                    ps_s = edgep.tile([1, 2 * NW], f32, tag="sm")
                    nc.tensor.matmul(ps_s[:], ones128[:], stats[:], start=True, stop=True)
                    ssum = miscp.tile([1, 4], f32, tag="ssum")
                    nc.vector.tensor_reduce(ssum[:, 0:1], ps_s[:, 0:NW], AX.X, ALU.add)
                    nc.vector.tensor_reduce(ssum[:, 1:2], ps_s[:, NW:2 * NW], AX.X, ALU.add)
                    nc.sync.dma_start(sdram[:, 0:2], ssum[:, 0:2])
                    nc.gpsimd.collective_compute(
                        "AllReduce", ALU.add, ins=[sdram[:]], outs=[sglob[:]],
                        replica_groups=RG)
                    gsum = miscp.tile([1, 8], f32, tag="gsum")
                    nc.sync.dma_start(gsum[:, 0:2], sglob[:, 0:2])
                    sc = miscp.tile([1, 8], f32, tag="sc")
                    inv_cnt = 1.0 / (float(N_NODES) * fout)
                    nc.vector.tensor_scalar_mul(sc[:, 0:2], gsum[:, 0:2], inv_cnt)
                    nc.vector.tensor_tensor(sc[:, 2:3], sc[:, 0:1], sc[:, 0:1], ALU.mult)
                    nc.vector.tensor_sub(sc[:, 3:4], sc[:, 1:2], sc[:, 2:3])
                    nc.vector.tensor_scalar_add(sc[:, 3:4], sc[:, 3:4], 1e-5)
                    nc.scalar.sqrt(sc[:, 4:5], sc[:, 3:4])
                    nc.vector.reciprocal(sc[:, 5:6], sc[:, 4:5])
                    mr = miscp.tile([1, 2], f32, tag="mr")
                    nc.vector.tensor_copy(mr[:, 0:1], sc[:, 0:1])
                    nc.vector.tensor_copy(mr[:, 1:2], sc[:, 5:6])
                    ps_b = edgep.tile([128, 2], f32, tag="sm")
                    nc.tensor.matmul(ps_b[:], onesT[:], mr[:], start=True, stop=True)
                    br = miscp.tile([128, 2], f32, tag="br")
                    nc.vector.tensor_copy(br[:], ps_b[:])
                    nfc = fout // 128
                    scl = miscp.tile([128, 32], f32, tag="scl")
                    bia = miscp.tile([128, 32], f32, tag="bia")
                    nc.vector.tensor_scalar(
                        scl[:, :nfc], lnw_t[li][:, :], br[:, 1:2], None, ALU.mult)
                    nc.vector.tensor_scalar(
                        bia[:, :nfc], scl[:, :nfc], br[:, 0:1], None, ALU.mult)
                    nc.vector.tensor_tensor(
                        bia[:, :nfc], lnb_t[li][:, :], bia[:, :nfc], ALU.subtract)
                    for q in range(nfc):
                        nc.vector.scalar_tensor_tensor(
                            zT[:, q, :], zT[:, q, :], scl[:, q:q + 1],
                            bia[:, q:q + 1].broadcast_to([128, NPAD]),
                            ALU.mult, ALU.add)
                    yT = zT
                    if debug:
                        nc.sync.dma_start(
                            dbg_zt[li][:], zT[:].rearrange("p q n -> p (q n)"))
                        nc.sync.dma_start(dbg_st[li][:], sc[:])
                edgeps.__exit__(None, None, None)

    nc.compile()
    _CACHE[key] = nc
    return nc


# ---------------------------------------------------------------- entry point
def make_in_maps(inputs):
    ncw, per_core = prep_edges(inputs["edge_index"])
    params = prep_params(inputs)
    alpha1 = prep_alpha1(per_core, ncw, params.pop("_al1"))
    in_maps = []
    for k in range(N_CORES):
        m = dict(params)
        m.update({kk: vv for kk, vv in per_core[k].items() if not kk.startswith("_")})
        m["alpha1"] = alpha1[k]
        in_maps.append(m)
    return ncw, in_maps


def kernel(**inputs):
    _install_ntff_hook()
    from concourse.bass_utils import run_bass_kernel_spmd

    ncw, in_maps = make_in_maps(inputs)
    nc = build(ncw)
    res = run_bass_kernel_spmd(nc, in_maps, core_ids=list(range(N_CORES)), trace=False)
    out = np.concatenate([res.results[k]["out"] for k in range(N_CORES)], axis=0)
    return out.astype(np.float32)


# revision 18
# speedup vs baseline: 1.2484x; 1.0031x over previous
"""GATNet (4-layer GAT, 10000 nodes / 50000 edges + self-loops) on 8 Trainium2 NeuronCores.

Self-contained: builds per-core shards on the host (edge bucketing by destination,
one-hot scatter masks, gather index tables), compiles one SPMD Bass program, runs it
on cores 0-7 via run_bass_kernel_spmd, and reassembles the full [10000, 1000] output.

Structure per layer:
  dense h = y @ W^T (bf16, attention projections folded as extra rhs columns)
  -> small AllGather of the per-node attention scores (al) + big AllGather of h;
     the whole softmax pre-phase (score gathers, e-values, segment denominators
     via one-hot matmuls, reciprocals, alphas) runs under the big AllGather
  -> gather phase: dma_gather of source h rows; one-hot scatter matmuls with the
     per-edge alpha folded into the stationary operand; bias via K=1 matmul
  -> ReLU eviction with fused row sums, graph-LayerNorm stats via tiny AllReduce,
     PE-transpose into feature-major for the next layer's lhsT.
Layer 1 never materializes h: by linearity sum_e alpha_e * (x W)[src_e] =
(sum_e alpha_e x[src_e]) W, so it scatters raw x rows (256 wide) and applies W1
once per destination window; its attention scores are computed exactly on the host.
"""
import sys
import types

import numpy as np
import ml_dtypes

BF16 = ml_dtypes.bfloat16

N_NODES = 10000
N_CORES = 8
NPC = 1250
NPAD = 1280
NT = 10
NW = 10
HEADS_L = [8, 8, 8, 1]
C_L = [448, 384, 256, 1000]
FIN_L = [256, 3584, 3072, 2048]
FOUT_L = [3584, 3072, 2048, 1000]
TCOL_L = [256, 3072, 2048, 1024]    # bf16 columns of the gather table (L1: raw x)
EXP_CLAMP = 35.0
DEN_TINY = 1e-30


def _install_ntff_hook():
    if "antenv.axon_hooks" in sys.modules:
        return
    try:
        import antenv
        from trn_agent_boot.trn_boot import _ntff_profile_via_ctypes
    except ImportError:
        return
    mod = types.ModuleType("antenv.axon_hooks")
    state = {"hook": None}
    mod.set_axon_ntff_profile_hook = lambda h: state.__setitem__("hook", h)
    mod.get_axon_ntff_profile_hook = lambda: state["hook"]
    sys.modules["antenv.axon_hooks"] = mod
    antenv.axon_hooks = mod
    mod.set_axon_ntff_profile_hook(_ntff_profile_via_ctypes("/opt/axon/libaxon_pjrt.so"))


# ---------------------------------------------------------------- host prep
def _table_row(n):
    return NPAD * (n // NPC) + (n % NPC)


def _wrap16(idx_chunk):
    w = idx_chunk.reshape(8, 16).T
    return np.tile(w, (8, 1)).astype(np.int16)


def prep_edges(edge_index):
    src = np.asarray(edge_index[0], dtype=np.int64)
    dst = np.asarray(edge_index[1], dtype=np.int64)
    src = np.concatenate([src, np.arange(N_NODES, dtype=np.int64)])
    dst = np.concatenate([dst, np.arange(N_NODES, dtype=np.int64)])

    buckets = [[[] for _ in range(NW)] for _ in range(N_CORES)]
    core_of = dst // NPC
    win_of = (dst % NPC) // 128
    order = np.argsort(dst, kind="stable")
    for e in order:
        buckets[core_of[e]][win_of[e]].append(e)

    ncw = []
    for w in range(NW):
        mx = max(len(buckets[k][w]) for k in range(N_CORES))
        ncw.append(max(1, -(-mx // 128)))
    nch = sum(ncw)

    per_core = []
    for k in range(N_CORES):
        idx_s = np.zeros((128, nch * 8), np.int16)
        idx_d = np.zeros((128, nch * 8), np.int16)
        mask = np.zeros((128, nch, 128), np.float32)
        maskT = np.zeros((128, nch, 128), np.float32)
        esrc = np.zeros((nch, 128), np.int64)
        edst = np.zeros((nch, 128), np.int64)
        ereal = np.zeros((nch, 128), bool)
        c0 = 0
        for w in range(NW):
            edges = buckets[k][w]
            for c in range(ncw[w]):
                part = edges[c * 128:(c + 1) * 128]
                srows = np.zeros(128, np.int64)
                drows = np.zeros(128, np.int64)
                for i, e in enumerate(part):
                    srows[i] = _table_row(src[e])
                    drows[i] = _table_row(dst[e])
                    esrc[c0 + c, i] = src[e]
                    edst[c0 + c, i] = dst[e]
                    ereal[c0 + c, i] = True
                    d_local = (dst[e] % NPC) - 128 * w
                    mask[i, c0 + c, d_local] = 1.0
                    maskT[d_local, c0 + c, i] = 1.0
                idx_s[:, (c0 + c) * 8:(c0 + c + 1) * 8] = _wrap16(srows)
                idx_d[:, (c0 + c) * 8:(c0 + c + 1) * 8] = _wrap16(drows)
            c0 += ncw[w]
        per_core.append(dict(
            idxs=idx_s, idxd=idx_d,
            mask=mask.reshape(128, nch * 128).astype(BF16),
            maskT=maskT.reshape(128, nch * 128).astype(BF16),
            _esrc=esrc, _edst=edst, _ereal=ereal,
        ))
    return tuple(ncw), per_core


def prep_alpha1(per_core, ncw, al1):
    """Exact layer-1 softmax on the host: alpha[e, h] per (chunk, slot)."""
    nch = sum(ncw)
    als = al1[:, :8].astype(np.float64)
    ald = al1[:, 8:].astype(np.float64)
    out = []
    for pc in per_core:
        esrc, edst, ereal = pc["_esrc"], pc["_edst"], pc["_ereal"]
        e = als[esrc] + ald[edst]                      # [nch, 128, 8]
        e = np.maximum(e, 0.2 * e)
        wv = np.exp(np.minimum(e, EXP_CLAMP)) * ereal[:, :, None]
        den = np.zeros((NPC, 8))
        np.add.at(den, (edst % NPC).reshape(-1), wv.reshape(-1, 8))
        alpha = wv / np.maximum(den[(edst % NPC)], 1e-300)
        # device layout: [part=slot, nch*8]
        return_arr = np.ascontiguousarray(
            alpha.transpose(1, 0, 2).reshape(128, nch * 8)).astype(BF16)
        out.append(return_arr)
    return out


def prep_params(inputs):
    p = {}
    x64 = np.asarray(inputs["x"], np.float64)
    al1 = None
    for li in range(4):
        H, C, fin, fout = HEADS_L[li], C_L[li], FIN_L[li], FOUT_L[li]
        W = np.asarray(inputs[f"W{li+1}"], np.float32)
        a_src = np.asarray(inputs[f"a_src{li+1}"], np.float32)
        a_dst = np.asarray(inputs[f"a_dst{li+1}"], np.float32)
        a_blk_s = np.zeros((fout, H), np.float32)
        a_blk_d = np.zeros((fout, H), np.float32)
        for h in range(H):
            a_blk_s[h * C:(h + 1) * C, h] = a_src[h]
            a_blk_d[h * C:(h + 1) * C, h] = a_dst[h]
        rhs = np.concatenate([W.T, W.T @ a_blk_s, W.T @ a_blk_d], axis=1)
        p[f"rhs{li+1}"] = np.ascontiguousarray(rhs).astype(BF16)
        p[f"brow{li+1}"] = np.asarray(inputs[f"b{li+1}"], np.float32).reshape(1, fout).astype(BF16)
        if li < 3:
            lw = np.asarray(inputs[f"ln{li+1}_w"], np.float32)
            lb = np.asarray(inputs[f"ln{li+1}_b"], np.float32)
            p[f"lnw{li+1}"] = np.ascontiguousarray(lw.reshape(fout // 128, 128).T)
            p[f"lnb{li+1}"] = np.ascontiguousarray(lb.reshape(fout // 128, 128).T)
        if li == 0:
            al1 = np.concatenate(
                [x64 @ (W.T @ a_blk_s).astype(np.float64),
                 x64 @ (W.T @ a_blk_d).astype(np.float64)], axis=1).astype(np.float32)
    # packed x table, replicated: [10240 rows, 768 bytes] = 512B x bf16 + 256B f32 al
    xtab = np.zeros((N_CORES * NPAD, 768), np.uint8)
    rows = _table_row(np.arange(N_NODES))
    xb = np.asarray(inputs["x"], np.float32).astype(BF16)
    xtab[rows, :512] = xb.view(np.uint8)
    alpad = np.zeros((N_NODES, 64), np.float32)
    alpad[:, :16] = al1
    xtab[rows, 512:768] = alpad.view(np.uint8)
    p["xtab"] = xtab.view(BF16)
    p["_al1"] = al1
    p["ident"] = np.eye(128, dtype=BF16)
    ones_b = np.zeros((1, NW * 128), np.float32)
    ones_b[0, :NPC] = 1.0
    p["ones_b"] = ones_b.astype(BF16)
    return p


# ---------------------------------------------------------------- device build
_CACHE = {}


def build(ncw, debug=False):
    key = (tuple(ncw), debug)
    if key in _CACHE:
        return _CACHE[key]

    import concourse.bacc as bacc
    import concourse.mybir as mybir
    import concourse.tile as tile
    from concourse.library_config import mlp

    f32 = mybir.dt.float32
    bf16 = mybir.dt.bfloat16
    i16 = mybir.dt.int16
    AX = mybir.AxisListType
    ALU = mybir.AluOpType
    ACTF = mybir.ActivationFunctionType

    nch = sum(ncw)
    ncmax = max(ncw)
    coff = [0]
    for w in range(NW):
        coff.append(coff[-1] + ncw[w])
    nc = bacc.Bacc("TRN2", num_swdge_queues=4)

    xtab_d = nc.declare_dram_parameter("xtab", [N_CORES * NPAD, 384], bf16, isOutput=False)
    rhs_d, brow_d, lnw_d, lnb_d = [], [], [], []
    for li in range(4):
        H, fout, fin = HEADS_L[li], FOUT_L[li], FIN_L[li]
        rhs_d.append(nc.declare_dram_parameter(f"rhs{li+1}", [fin, fout + 2 * H], bf16, isOutput=False))
        brow_d.append(nc.declare_dram_parameter(f"brow{li+1}", [1, fout], bf16, isOutput=False))
        if li < 3:
            lnw_d.append(nc.declare_dram_parameter(f"lnw{li+1}", [128, fout // 128], f32, isOutput=False))
            lnb_d.append(nc.declare_dram_parameter(f"lnb{li+1}", [128, fout // 128], f32, isOutput=False))
    alpha1_d = nc.declare_dram_parameter("alpha1", [128, nch * 8], bf16, isOutput=False)
    idxs_d = nc.declare_dram_parameter("idxs", [128, nch * 8], i16, isOutput=False)
    idxd_d = nc.declare_dram_parameter("idxd", [128, nch * 8], i16, isOutput=False)
    mask_d = nc.declare_dram_parameter("mask", [128, nch * 128], bf16, isOutput=False)
    maskT_d = nc.declare_dram_parameter("maskT", [128, nch * 128], bf16, isOutput=False)
    ones_d = nc.declare_dram_parameter("ones_b", [1, NW * 128], bf16, isOutput=False)
    ident_d = nc.declare_dram_parameter("ident", [128, 128], bf16, isOutput=False)
    out_d = nc.declare_dram_parameter("out", [NPC, 1000], f32, isOutput=True)
    dbg_zt, dbg_st = [], []
    if debug:
        for li in range(3):
            dbg_zt.append(nc.declare_dram_parameter(
                f"dbg_zt{li}", [128, 28 * NPAD], bf16, isOutput=True))
            dbg_st.append(nc.declare_dram_parameter(
                f"dbg_st{li}", [1, 8], f32, isOutput=True))

    RG = [list(range(N_CORES))]

    with tile.TileContext(nc) as tc:
        with (
            tc.tile_pool(name="const", bufs=1) as constp,
            tc.tile_pool(name="yt", bufs=1) as ytp,
            tc.tile_pool(name="rhs", bufs=5) as rhsp,
            tc.tile_pool(name="stage", bufs=2) as stagep,
            tc.tile_pool(name="gath", bufs=2) as gathp,
            tc.tile_pool(name="mw", bufs=2) as mwp,
            tc.tile_pool(name="eph", bufs=2) as ephp,
            tc.tile_pool(name="z", bufs=2) as zp,
            tc.tile_pool(name="misc", bufs=2) as miscp,
            tc.tile_pool(name="dram", bufs=1, space="DRAM") as dram,
        ):
            nc.gpsimd.load_library(mlp)

            idxs_t = constp.tile([128, nch, 8], i16, tag="idxs")
            nc.sync.dma_start(idxs_t[:], idxs_d[:].rearrange("p (c d) -> p c d", c=nch))
            idxd_t = constp.tile([128, nch, 8], i16, tag="idxd")
            nc.sync.dma_start(idxd_t[:], idxd_d[:].rearrange("p (c d) -> p c d", c=nch))
            ones_t = constp.tile([1, NW, 128], bf16, tag="onesb")
            nc.sync.dma_start(ones_t[:], ones_d[:].rearrange("p (w d) -> p w d", w=NW))
            ident_t = constp.tile([128, 128], bf16, tag="ident")
            nc.sync.dma_start(ident_t[:], ident_d[:])
            ones128 = constp.tile([128, 1], f32, tag="ones128")
            nc.vector.memset(ones128[:], 1.0)
            onesT = constp.tile([1, 128], f32, tag="onesT")
            nc.vector.memset(onesT[:], 1.0)
            lnw_t, lnb_t = [], []
            for li in range(3):
                t1 = constp.tile([128, FOUT_L[li] // 128], f32, tag=f"lnw{li}")
                nc.sync.dma_start(t1[:], lnw_d[li][:])
                t2 = constp.tile([128, FOUT_L[li] // 128], f32, tag=f"lnb{li}")
                nc.sync.dma_start(t2[:], lnb_d[li][:])
                lnw_t.append(t1)
                lnb_t.append(t2)
            alpha1_t = constp.tile([128, nch, 8], bf16, tag="alpha1")
            nc.sync.dma_start(alpha1_t[:], alpha1_d[:].rearrange("p (c d) -> p c d", c=nch))
            # resident W1 rhs (small; needed per destination window in layer 1)
            rt1 = constp.tile([128, 2, 3584], bf16, tag="rt1")
            nc.sync.dma_start(
                rt1[:], rhs_d[0][:, 0:3584].rearrange("(k p) n -> p k n", p=128))

            yT = None  # produced by each layer's LN for the next layer

            for li in range(4):
                H, C, fin, fout = HEADS_L[li], C_L[li], FIN_L[li], FOUT_L[li]
                tcol = TCOL_L[li]
                kch = fin // 128
                acols = 2 * H

                brow = constp.tile([1, 3584], bf16, tag="brow")
                nc.sync.dma_start(brow[:, :fout], brow_d[li][:])

                if li > 0:
                    # ===== dense: h = y @ W^T (+ al columns); al chunk FIRST so the
                    # small al AllGather + softmax pre-phase overlap the dense phase
                    half = {3072: 2048, 2048: 1024, 1024: 512}[tcol]
                    shard_a = dram.tile([NPAD, half], bf16, tag=f"sharda{li}")
                    shard_b = dram.tile([NPAD, tcol - half], bf16, tag=f"shardb{li}")
                    glob_a = dram.tile([N_CORES * NPAD, half], bf16, addr_space="Shared", tag=f"globa{li}")
                    glob_b = dram.tile([N_CORES * NPAD, tcol - half], bf16, addr_space="Shared", tag=f"globb{li}")
                    ashard = dram.tile([NPAD, 64], f32, tag=f"ashard{li}")
                    aglob = dram.tile([N_CORES * NPAD, 64], f32, addr_space="Shared", tag=f"aglob{li}")
                    fcs = [(fout, acols)]
                    o = 0
                    while o < fout:
                        w_ = min(512, fout - o)
                        fcs.append((o, w_))
                        o += w_
                    with tc.tile_pool(name=f"psA{li}", bufs=2, space="PSUM") as mmp:
                        for fci, (fo, fw) in enumerate(fcs):
                            kgrps = [(k0, min(7, kch - k0)) for k0 in range(0, kch, 7)]
                            rts = []
                            for (k0, kn) in kgrps:
                                rt = rhsp.tile([128, 7, 512], bf16, tag="rhs")
                                nc.sync.dma_start(
                                    rt[:, :kn, :fw],
                                    rhs_d[li][k0 * 128:(k0 + kn) * 128, fo:fo + fw]
                                    .rearrange("(k p) n -> p k n", p=128))
                                rts.append(rt)
                            for t in range(NT):
                                ps = mmp.tile([128, 512], f32, tag="mm")
                                for kc in range(kch):
                                    nc.tensor.matmul(
                                        ps[:, :fw],
                                        yT[:, kc, t * 128:(t + 1) * 128],
                                        rts[kc // 7][:, kc % 7, :fw],
                                        start=(kc == 0),
                                        stop=(kc == kch - 1))
                                hw = max(0, min(fw, fout - fo))
                                if hw > 0:
                                    st = stagep.tile([128, 512], bf16, tag="stg")
                                    if t % 2 == 0:
                                        nc.scalar.activation(st[:, :hw], ps[:, :hw], ACTF.Copy)
                                    else:
                                        nc.vector.tensor_copy(st[:, :hw], ps[:, :hw])
                                    if fo < half:
                                        nc.sync.dma_start(
                                            shard_a[t * 128:(t + 1) * 128, fo:fo + hw], st[:, :hw])
                                    else:
                                        nc.sync.dma_start(
                                            shard_b[t * 128:(t + 1) * 128, fo - half:fo - half + hw],
                                            st[:, :hw])
                                if hw < fw:
                                    a0 = fo + hw - fout
                                    sa = stagep.tile([128, 64], f32, tag="stga")
                                    nc.vector.tensor_copy(sa[:, :fw - hw], ps[:, hw:fw])
                                    nc.sync.dma_start(
                                        ashard[t * 128:(t + 1) * 128, a0:a0 + fw - hw],
                                        sa[:, :fw - hw])
                            if fci == 0:
                                nc.gpsimd.collective_compute(
                                    "AllGather", ALU.bypass, ins=[ashard[:]], outs=[aglob[:]],
                                    replica_groups=RG)
                            if fci == half // 512:
                                nc.gpsimd.collective_compute(
                                    "AllGather", ALU.bypass, ins=[shard_a[:]], outs=[glob_a[:]],
                                    replica_groups=RG)

                        # ---- softmax pre-phase, overlapping the dense phase
                        al_f32 = aglob[:]
                        al_step = 64
                        alpha_all = ephp.tile([128, nch, 8], bf16, tag="alpha", bufs=1)
                        for w in range(NW):
                            c0, ncwW = coff[w], ncw[w]
                            ne = ncwW * 128
                            As = ephp.tile([128, ncmax, 64], f32, tag="as", bufs=1)
                            Ad = ephp.tile([128, ncmax, 64], f32, tag="ad", bufs=1)
                            nc.gpsimd.dma_gather(
                                As[:, :ncwW, :], al_f32,
                                idxs_t[:, c0:c0 + ncwW, :], ne, ne, 64, elem_step=al_step,
                                queue_num=(2 * w) % 4)
                            nc.gpsimd.dma_gather(
                                Ad[:, :ncwW, :], al_f32,
                                idxd_t[:, c0:c0 + ncwW, :], ne, ne, 64, elem_step=al_step,
                                queue_num=(2 * w + 1) % 4)
                            maskw = mwp.tile([128, ncmax, 128], bf16, tag="mw")
                            nc.scalar.dma_start(
                                maskw[:, :ncwW, :],
                                mask_d[:, c0 * 128:(c0 + ncwW) * 128].rearrange(
                                    "p (c d) -> p c d", c=ncwW))
                            maskTw = mwp.tile([128, ncmax, 128], bf16, tag="mwT")
                            nc.scalar.dma_start(
                                maskTw[:, :ncwW, :],
                                maskT_d[:, c0 * 128:(c0 + ncwW) * 128].rearrange(
                                    "p (c d) -> p c d", c=ncwW))

                            ev = ephp.tile([128, ncmax, 8], f32, tag="ev")
                            nc.vector.tensor_tensor(
                                ev[:, :ncwW, :H], As[:, :ncwW, 0:H], Ad[:, :ncwW, H:2 * H], ALU.add)
                            nc.vector.scalar_tensor_tensor(
                                ev[:, :ncwW, :H], ev[:, :ncwW, :H], 0.2, ev[:, :ncwW, :H],
                                ALU.mult, ALU.max)
                            nc.vector.tensor_scalar_min(ev[:, :ncwW, :H], ev[:, :ncwW, :H], EXP_CLAMP)
                            wv = ephp.tile([128, ncmax, 8], bf16, tag="wv")
                            nc.scalar.activation(wv[:, :ncwW, :H], ev[:, :ncwW, :H], ACTF.Exp)

                            ps_den = mmp.tile([128, 8], f32, tag="smA")
                            for c in range(ncwW):
                                nc.tensor.matmul(
                                    ps_den[:, :H], maskw[:, c, :], wv[:, c, :H],
                                    start=(c == 0), stop=(c == ncwW - 1))
                            rden_f = ephp.tile([128, 8], f32, tag="rdenf")
                            nc.vector.tensor_scalar_max(rden_f[:, :H], ps_den[:, :H], DEN_TINY)
                            rden2 = ephp.tile([128, 8], f32, tag="rden2")
                            nc.vector.reciprocal(rden2[:, :H], rden_f[:, :H])
                            rden = ephp.tile([128, 8], bf16, tag="rden")
                            nc.vector.tensor_copy(rden[:, :H], rden2[:, :H])
                            for c in range(ncwW):
                                ps_exp = mmp.tile([128, 8], f32, tag="smA")
                                nc.tensor.matmul(
                                    ps_exp[:, :H], maskTw[:, c, :], rden[:, :H],
                                    start=True, stop=True)
                                nc.vector.tensor_tensor(
                                    alpha_all[:, c0 + c, :H], wv[:, c, :H], ps_exp[:, :H], ALU.mult)

                    # second-half table AllGather (emitted after pre-phase so the
                    # gpsimd queue isn't blocked behind its completion wait)
                    nc.gpsimd.collective_compute(
                        "AllGather", ALU.bypass, ins=[shard_b[:]], outs=[glob_b[:]],
                        replica_groups=RG)
                else:
                    alpha_all = alpha1_t

                # ===== edge phase
                edgeps = tc.tile_pool(name=f"psB{li}", bufs=1, space="PSUM")
                edgep = edgeps.__enter__()
                if li < 3:
                    zT = ytp.tile([128, 28, NPAD], bf16, tag="yt")
                    stats = miscp.tile([128, 2 * NW], f32, tag="stats")

                # ---- gather + scatter phase
                for w in range(NW):
                    c0, ncwW = coff[w], ncw[w]
                    maskw = mwp.tile([128, ncmax, 128], bf16, tag="mw")
                    nc.scalar.dma_start(
                        maskw[:, :ncwW, :],
                        mask_d[:, c0 * 128:(c0 + ncwW) * 128].rearrange(
                            "p (c d) -> p c d", c=ncwW))

                    if li == 0:
                        ps_agg = edgep.tile([128, 2, 8, 128], f32, tag="out")
                    else:
                        ps_out = edgep.tile([128, fout], f32, tag="out")

                    for cp in range(0, ncwW, 2):
                        cw = min(2, ncwW - cp)
                        if li == 0:
                            G = gathp.tile([128, 2, 256], bf16, tag="G")
                            nc.gpsimd.dma_gather(
                                G[:, :cw, :], xtab_d[:][:, 0:256],
                                idxs_t[:, c0 + cp:c0 + cp + cw, :], cw * 128, cw * 128,
                                256, elem_step=384, queue_num=(cp // 2) % 4)
                        else:
                            G = gathp.tile([128, 2, half], bf16, tag="G")
                            nc.gpsimd.dma_gather(
                                G[:, :cw, :], glob_a[:],
                                idxs_t[:, c0 + cp:c0 + cp + cw, :], cw * 128, cw * 128,
                                half, elem_step=half, queue_num=(cp // 2) % 4)
                            Gb = gathp.tile([128, 2, tcol - half], bf16, tag="Gb")
                            nc.gpsimd.dma_gather(
                                Gb[:, :cw, :], glob_b[:],
                                idxs_t[:, c0 + cp:c0 + cp + cw, :], cw * 128, cw * 128,
                                tcol - half, elem_step=tcol - half,
                                queue_num=(cp // 2 + 2) % 4)
                        for c in range(cp, cp + cw):
                            lhs = ephp.tile([128, 8, 128], bf16, tag="lhs")
                            nc.vector.tensor_tensor(
                                lhs[:, :H, :],
                                maskw[:, c, :].unsqueeze(1).broadcast_to([128, H, 128]),
                                alpha_all[:, c0 + c, :H].unsqueeze(2).broadcast_to([128, H, 128]),
                                ALU.mult)
                            if li == 0:
                                for kc in range(2):
                                    for hg in range(2):
                                        nc.tensor.matmul(
                                            ps_agg[:, kc, hg * 4:(hg + 1) * 4, :],
                                            G[:, c - cp, kc * 128:(kc + 1) * 128],
                                            lhs[:, hg * 4:(hg + 1) * 4, :],
                                            start=(c == 0), stop=False)
                            else:
                                o = 0
                                while o < fout:
                                    h = o // C
                                    e = min((h + 1) * C, (o // 512 + 1) * 512, fout)
                                    if o < half:
                                        rhs_g = G[:, c - cp, o:e]
                                    else:
                                        rhs_g = Gb[:, c - cp, o - half:e - half]
                                    nc.tensor.matmul(
                                        ps_out[:, o:e], lhs[:, h, :], rhs_g,
                                        start=(c == 0 and o % 512 == 0), stop=False)
                                    o = e

                    if li == 0:
                        # xaggT came out of the scatter matmuls already transposed
                        xs = zp.tile([128, 2, 8, 128], bf16, tag="xs", bufs=1)
                        nc.vector.tensor_copy(xs[:], ps_agg[:])
                        ps_out = edgep.tile([128, fout], f32, tag="out")
                        for h in range(H):
                            o = h * C
                            while o < (h + 1) * C:
                                e = min((o // 512 + 1) * 512, (h + 1) * C)
                                for kc in range(2):
                                    nc.tensor.matmul(
                                        ps_out[:, o:e], xs[:, kc, h, :],
                                        rt1[:, kc, o:e],
                                        start=(kc == 0 and o % 512 == 0), stop=False)
                                o = e
                    # bias add
                    o = 0
                    while o < fout:
                        e = min(o + 512, fout)
                        nc.tensor.matmul(
                            ps_out[:, o:e], ones_t[:, w, :], brow[:, o:e],
                            start=False, stop=(e == fout))
                        o = e

                    if li < 3:
                        z = zp.tile([128, 3584], bf16, tag="z")
                        nc.scalar.activation(
                            z[:, :fout], ps_out[:, :fout], ACTF.Relu,
                            accum_out=stats[:, w:w + 1])
                        sq = zp.tile([128, 3584], bf16, tag="z")
                        nc.vector.scalar_tensor_tensor(
                            sq[:, :fout], z[:, :fout], 1.0, z[:, :fout],
                            ALU.mult, ALU.mult,
                            accum_out=stats[:, NW + w:NW + w + 1])
                        for q in range(0, fout // 128, 4):
                            qn = min(4, fout // 128 - q)
                            ps_t = edgep.tile([128, 4, 128], bf16, tag="sm")
                            for j in range(qn):
                                nc.tensor.matmul(
                                    ps_t[:, j, :], z[:, (q + j) * 128:(q + j + 1) * 128],
                                    ident_t[:], is_transpose=True,
                                    start=(j == 0), stop=(j == qn - 1))
                            if (q // 4) % 2 == 0:
                                nc.vector.tensor_copy(
                                    zT[:, q:q + qn, w * 128:(w + 1) * 128],
                                    ps_t[:, :qn, :])
                            else:
                                nc.scalar.activation(
                                    zT[:, q:q + qn, w * 128:(w + 1) * 128],
                                    ps_t[:, :qn, :], ACTF.Copy)
                    else:
                        zf = zp.tile([128, 1024], f32, tag="z")
                        nc.scalar.activation(zf[:, :fout], ps_out[:, :fout], ACTF.Copy)
                        rows = min(128, NPC - w * 128)
                        nc.sync.dma_start(out_d[w * 128:w * 128 + rows, :], zf[:rows, :fout])

                # ===== graph LayerNorm + next yT
                if li < 3:
                    sdram = dram.tile([1, 64], f32, tag=f"sd{li}")
                    sglob = dram.tile([1, 64], f32, addr_space="Shared", tag=f"sg{li}")
                    ps_s = edgep.tile([1, 2 * NW], f32, tag="sm")
                    nc.tensor.matmul(ps_s[:], ones128[:], stats[:], start=True, stop=True)
                    ssum = miscp.tile([1, 4], f32, tag="ssum")
                    nc.vector.tensor_reduce(ssum[:, 0:1], ps_s[:, 0:NW], AX.X, ALU.add)
                    nc.vector.tensor_reduce(ssum[:, 1:2], ps_s[:, NW:2 * NW], AX.X, ALU.add)
                    nc.sync.dma_start(sdram[:, 0:2], ssum[:, 0:2])
                    nc.gpsimd.collective_compute(
                        "AllReduce", ALU.add, ins=[sdram[:]], outs=[sglob[:]],
                        replica_groups=RG)
                    gsum = miscp.tile([1, 8], f32, tag="gsum")
                    nc.sync.dma_start(gsum[:, 0:2], sglob[:, 0:2])
                    sc = miscp.tile([1, 8], f32, tag="sc")
                    inv_cnt = 1.0 / (float(N_NODES) * fout)
                    nc.vector.tensor_scalar_mul(sc[:, 0:2], gsum[:, 0:2], inv_cnt)
                    nc.vector.tensor_tensor(sc[:, 2:3], sc[:, 0:1], sc[:, 0:1], ALU.mult)
                    nc.vector.tensor_sub(sc[:, 3:4], sc[:, 1:2], sc[:, 2:3])
                    nc.vector.tensor_scalar_add(sc[:, 3:4], sc[:, 3:4], 1e-5)
                    nc.scalar.sqrt(sc[:, 4:5], sc[:, 3:4])
                    nc.vector.reciprocal(sc[:, 5:6], sc[:, 4:5])
                    mr = miscp.tile([1, 2], f32, tag="mr")
                    nc.vector.tensor_copy(mr[:, 0:1], sc[:, 0:1])
                    nc.vector.tensor_copy(mr[:, 1:2], sc[:, 5:6])
                    ps_b = edgep.tile([128, 2], f32, tag="sm")
                    nc.tensor.matmul(ps_b[:], onesT[:], mr[:], start=True, stop=True)
                    br = miscp.tile([128, 2], f32, tag="br")
                    nc.vector.tensor_copy(br[:], ps_b[:])
                    nfc = fout // 128
                    scl = miscp.tile([128, 32], f32, tag="scl")
                    bia = miscp.tile([128, 32], f32, tag="bia")
                    nc.vector.tensor_scalar(
                        scl[:, :nfc], lnw_t[li][:, :], br[:, 1:2], None, ALU.mult)
                    nc.vector.tensor_scalar(
                        bia[:, :nfc], scl[:, :nfc], br[:, 0:1], None, ALU.mult)
                    nc.vector.tensor_tensor(
                        bia[:, :nfc], lnb_t[li][:, :], bia[:, :nfc], ALU.subtract)
                    for q in range(nfc):
                        nc.vector.scalar_tensor_tensor(
                            zT[:, q, :], zT[:, q, :], scl[:, q:q + 1],
                            bia[:, q:q + 1].broadcast_to([128, NPAD]),
                            ALU.mult, ALU.add)
                    yT = zT
                    if debug:
                        nc.sync.dma_start(
                            dbg_zt[li][:], zT[:].rearrange("p q n -> p (q n)"))
                        nc.sync.dma_start(dbg_st[li][:], sc[:])
                edgeps.__exit__(None, None, None)

    nc.compile()
    _CACHE[key] = nc
    return nc


# ---------------------------------------------------------------- entry point
def make_in_maps(inputs):
    ncw, per_core = prep_edges(inputs["edge_index"])
    params = prep_params(inputs)
    alpha1 = prep_alpha1(per_core, ncw, params.pop("_al1"))
    in_maps = []
    for k in range(N_CORES):
        m = dict(params)
        m.update({kk: vv for kk, vv in per_core[k].items() if not kk.startswith("_")})
        m["alpha1"] = alpha1[k]
        in_maps.append(m)
    return ncw, in_maps


def kernel(**inputs):
    _install_ntff_hook()
    from concourse.bass_utils import run_bass_kernel_spmd

    ncw, in_maps = make_in_maps(inputs)
    nc = build(ncw)
    res = run_bass_kernel_spmd(nc, in_maps, core_ids=list(range(N_CORES)), trace=False)
    out = np.concatenate([res.results[k]["out"] for k in range(N_CORES)], axis=0)
    return out.astype(np.float32)
